# revision 14
# baseline (speedup 1.0000x reference)
"""CapsNet Trainium2 kernel: 8-core SPMD Bass/Tile implementation.

Strategy (v3):
  Phase 1 (contraction-parallel): the dominant op is
     dct_emb = relu(norm(log|DCT|) @ W_emb.T + b_emb),  [512,102400]x[102400,768]
  Each core owns a 12800-wide slice of the 102400 contraction dim
  (~128us of bf16 PE time — the roofline). log|x| is precomputed on
  host (needed there for the global mean/std anyway) and the affine
  normalization folds into the matmul epilogue. Phase-1 streaming loads
  own the Sync DMA queue; constant loads ride other engine queues.

  Phase 2 (batch-parallel): the per-core partial G^T[768,512] products
  are combined with a bf16 ReduceScatter over the batch dim (DRAM
  layout [8, 768, 64] so rank r receives exactly its 64 batch columns).
  Each core runs primary caps + dynamic routing for its own 64 rows;
  the two batch-mean agreement reductions are tiny [2,192] AllReduces.
  The host concatenates the per-core [64,128] outputs.

  Dynamic routing never materializes u_hat[B,192,2,64]:
     s_c   = (W2 * c)^T @ u           (contraction over (r,i)=1536)
     P_c   = W3_c^T @ v_c             (back-projected v)
     a_rc  = sum_b sum_i u * P_c      (agreement, mul+reduce)
  The elementwise digit squash uses s*|s|/(1+s^2) (the 1e-7 eps only
  matters at |s|~3e-4 where the output is ~1e-11 — far below
  tolerance), so routing needs no scalar-engine table switches.
"""

import os
import sys

import numpy as np

if "/opt/trn_rl_repo" not in sys.path:
    sys.path.insert(0, "/opt/trn_rl_repo")

import concourse.bass as bass  # noqa: E402
import concourse.mybir as mybir  # noqa: E402
import concourse.tile as tile  # noqa: E402
from concourse import bacc  # noqa: E402
from concourse.bass_utils import run_bass_kernel_spmd  # noqa: E402
from concourse.masks import make_identity  # noqa: E402

try:
    import ml_dtypes  # noqa: E402

    _BF16 = ml_dtypes.bfloat16
except Exception:  # pragma: no cover
    _BF16 = None

N_CORES = 8
B = 512  # global batch
BL = B // N_CORES  # 64 local batch rows per core
K, KC = 102400, 12800  # contraction dim, per-core slice
E = 768  # embedding
ET = E // 128  # e chunks
KT = KC // 128  # k tiles per core (100)
GROUP = 5  # k tiles per load DMA
RI = 1536  # (route, in_cap) flat = 192*8
RT = RI // 128  # 12 tiles
NCLS = 2
OC = 64  # out caps channels
F32 = mybir.dt.float32
BF = mybir.dt.bfloat16

PHASE1_DT = os.environ.get("CAPS_P1_DT", "bf16")
RS_DT = os.environ.get("CAPS_RS_DT", "bf16")
# bisection: 1=phase1 only, 2=+RS, 3=+prim/squash/u2, 5=full
STOP = int(os.environ.get("CAPS_STOP", "5"))

_CACHE = {}


def _emit(nc, tc, const, loads, work, dram, io):
    dt1 = BF if PHASE1_DT == "bf16" else F32
    dt_rs = BF if RS_DT == "bf16" else F32
    rg = [list(range(N_CORES))]
    dlog_t, wp, beta, img_t, capt_t, wm2, bias3, w2, w3, y = io

    def debug_out(fill=None):
        out_sb = work.tile([BL, 128], F32, tag="outsb", name="outsb")
        nc.vector.memset(out_sb[:], 0.0)
        if fill is not None:
            fill(out_sb)
        nc.sync.dma_start(y[:, :], out_sb[:])

    # ---------------- phase 1: big matmul (loads on Sync queue) --------
    psA_cm = tc.tile_pool(name="psA", bufs=1, space="PSUM")
    psA = psA_cm.__enter__()
    g_ps = [psA.tile([128, B], F32, tag=f"g{ec}", name=f"g{ec}") for ec in range(ET)]
    n_loads = KT // GROUP
    late_loads = []  # (dst_tile_ap, src_ap) deferred const loads

    def _emit_phase1():
        li_first_late = 3
        for li in range(n_loads):
            k0 = li * GROUP * 128
            dlog = loads.tile([128, GROUP, B], dt1, tag="dlog")
            nc.sync.dma_start(
                dlog[:],
                dlog_t[k0 : k0 + GROUP * 128, :].rearrange(
                    "(s p) b -> p s b", p=128
                ),
            )
            w_tile = loads.tile([128, GROUP, E], dt1, tag="w")
            nc.sync.dma_start(
                w_tile[:],
                wp[k0 : k0 + GROUP * 128, :].rearrange("(s p) e -> p s e", p=128),
            )
            if li == 6:
                # gate: delay const loads until mid-phase-1 so they don't
                # compete with the streaming ramp for DMA bandwidth
                gate_s = work.tile([1, 8], dt1, tag="gate_s")
                nc.scalar.copy(gate_s[:], dlog[:1, 0, :8])
                gate_g = work.tile([1, 8], dt1, tag="gate_g")
                nc.gpsimd.tensor_copy(gate_g[:], dlog[:1, 0, :8])
                for i_, (d, s_) in enumerate(late_loads):
                    eng = nc.scalar if i_ % 2 == 0 else nc.gpsimd
                    eng.dma_start(d, s_)
                late_loads.clear()
            for s in range(GROUP):
                kt = li * GROUP + s
                for ec in range(ET):
                    nc.tensor.matmul(
                        g_ps[ec][:],
                        w_tile[:, s, ec * 128 : (ec + 1) * 128],
                        dlog[:, s, :],
                        start=(kt == 0),
                        stop=(kt == KT - 1),
                    )

    # ---------------- constants (DMA on vector/scalar/tensor queues) ----
    eps_sq = const.tile([128, 1], F32)
    nc.vector.memset(eps_sq[:], 1e-7)
    ident_bf = const.tile([128, 128], BF)
    make_identity(nc, ident_bf[:])
    ident_f = const.tile([128, 128], F32)
    make_identity(nc, ident_f[:])
    beta_sb = const.tile([128, ET], F32)
    late_loads.append((beta_sb[:], beta[:].rearrange("(t p) -> p t", p=128)))
    emb_sb = {}  # (m, et) -> [128, BL] bf16 tile (lhsT source for prim)
    for m, src in ((0, img_t), (1, capt_t)):
        for et in range(ET):
            t = const.tile([128, BL], BF, tag=f"emb{m}_{et}", name=f"emb{m}_{et}")
            late_loads.append((t[:], src[et * 128 : (et + 1) * 128, :]))
            emb_sb[(m, et)] = t
    wm2_sb = {}
    for m in range(3):
        for et in range(ET):
            t = const.tile([128, 512], BF, tag=f"wm2_{m}_{et}", name=f"wm2_{m}_{et}")
            late_loads.append((t[:], wm2[m, et * 128 : (et + 1) * 128, :]))
            wm2_sb[(m, et)] = t
    bias_sb = []
    for m in range(3):
        t = const.tile([1, 512], BF, tag=f"bias{m}", name=f"bias{m}")
        late_loads.append((t[:], bias3[m : m + 1, :]))
        bias_sb.append(t)
    ones1 = const.tile([1, BL], BF)
    nc.vector.memset(ones1[:], 1.0)
    w2_sb = []
    for t_ in range(RT):
        t = const.tile([128, 128], BF, tag=f"w2_{t_}", name=f"w2_{t_}")
        late_loads.append((t[:], w2[t_ * 128 : (t_ + 1) * 128, :]))
        w2_sb.append(t)
    w3_sb = []
    for c in range(NCLS):
        t = const.tile([OC, RI], BF, tag=f"w3_{c}", name=f"w3_{c}")
        late_loads.append((t[:], w3[c * OC : (c + 1) * OC, :]))
        w3_sb.append(t)

    _emit_phase1()

    # evacuate PSUM -> SBUF -> cc_in[8, 768, 64]; ReduceScatter over batch
    cc_in = dram.tile([N_CORES, E, BL], dt_rs)
    for ec in range(ET):
        g_sb = work.tile([128, B], dt_rs, tag="gsb", bufs=2)
        if ec % 2 == 0:
            nc.vector.tensor_copy(g_sb[:], g_ps[ec][:])
        else:
            nc.scalar.copy(g_sb[:], g_ps[ec][:])
        nc.sync.dma_start(
            cc_in[:, ec * 128 : (ec + 1) * 128, :].rearrange("r p b -> p r b"),
            g_sb[:].rearrange("p (r b) -> p r b", r=N_CORES),
        )
    psA_cm.__exit__(None, None, None)
    ps1_cm = tc.tile_pool(name="psB", bufs=1, space="PSUM")
    ps1 = ps1_cm.__enter__()
    if STOP == 1:
        debug_out()
        return

    rs_out = dram.tile([E, BL], dt_rs)
    nc.gpsimd.collective_compute(
        "ReduceScatter",
        mybir.AluOpType.add,
        replica_groups=rg,
        ins=[cc_in[:]],
        outs=[rs_out[:]],
    )
    if STOP == 2:
        debug_out(lambda o: nc.sync.dma_start(o[:, :64], rs_out[:BL, :]))
        return

    # ---------------- primary caps + squash + transpose to u2 ----------
    u2_all = const.tile([128, RT, BL], BF)  # [(r,i)-tile, t, b]

    def prim_chain(m):
        pm = ps1.tile([BL, 512], F32, tag="pp", bufs=2, name="pm")
        for et in range(ET):
            nc.tensor.matmul(
                pm[:],
                emb_sb[(m, et)][:],
                wm2_sb[(m, et)][:],
                start=(et == 0),
                stop=False,
            )
        nc.tensor.matmul(pm[:], ones1[:], bias_sb[m][:], start=False, stop=True)
        upre_c = work.tile([BL, 512], F32, tag="upre", bufs=3, name="upre")
        nc.vector.tensor_copy(upre_c[:], pm[:])
        # squash over i (groups of 8 in the free dim), 64 routes here
        sq8 = work.tile([BL, 512], F32, tag="sq8", bufs=2)
        nc.vector.tensor_mul(sq8[:], upre_c[:], upre_c[:])
        usq = work.tile([BL, 64], F32, tag="usq")
        nc.vector.tensor_reduce(
            usq[:],
            sq8[:].rearrange("p (r i) -> p r i", i=8),
            axis=mybir.AxisListType.X,
            op=mybir.AluOpType.add,
        )
        t1 = work.tile([BL, 64], F32, tag="fa")
        nc.scalar.activation(
            t1[:], usq[:], mybir.ActivationFunctionType.Sqrt, bias=eps_sq[:BL, :]
        )
        t2 = work.tile([BL, 64], F32, tag="fb")
        nc.vector.tensor_scalar_add(t2[:], usq[:], 1.0)
        t3 = work.tile([BL, 64], F32, tag="fc")
        nc.vector.tensor_mul(t3[:], t1[:], t2[:])
        t4 = work.tile([BL, 64], F32, tag="fd")
        nc.vector.reciprocal_approx_fast(t4[:], t3[:])
        t5 = work.tile([BL, 64], F32, tag="fe")
        nc.vector.tensor_mul(t5[:], t4[:], usq[:])
        usquash = work.tile([BL, 512], BF, tag="usquash", bufs=2)
        nc.vector.tensor_tensor(
            usquash[:].rearrange("p (r i) -> p r i", i=8),
            upre_c[:].rearrange("p (r i) -> p r i", i=8),
            t5[:].broadcast_to([BL, 64, 8]),
            op=mybir.AluOpType.mult,
        )
        for j in range(4):
            tp = ps1.tile([128, BL], BF, tag="pp", bufs=2, name="tp")
            nc.tensor.transpose(
                tp[:], usquash[:, j * 128 : (j + 1) * 128], ident_bf[:BL, :BL]
            )
            nc.vector.tensor_copy(u2_all[:, 4 * m + j, :], tp[:])

    # img/capt chains execute on the PE during the ReduceScatter
    prim_chain(0)
    prim_chain(1)

    # iter-0 s-matmul partial accumulation over img/capt tiles (RS shadow)
    s_ps = [
        ps1.tile([OC, BL], F32, tag=f"sps{c}", bufs=1, name=f"sps{c}")
        for c in range(NCLS)
    ]
    for c in range(NCLS):
        for t_ in range(8):
            nc.tensor.matmul(
                s_ps[c][:],
                w2_sb[t_][:, c * OC : (c + 1) * OC],
                u2_all[:, t_, :],
                start=(t_ == 0),
                stop=False,
            )

    # dct embedding tiles: relu(rs_out + beta) -> bf16 (vector, no tables)
    gp = work.tile([128, ET, BL], dt_rs, tag="gp")
    nc.sync.dma_start(gp[:], rs_out[:].rearrange("(t p) b -> p t b", p=128))
    for et in range(ET):
        t = const.tile([128, BL], BF, tag=f"emb2_{et}", name=f"emb2_{et}")
        nc.vector.tensor_scalar(
            t[:],
            gp[:, et, :],
            beta_sb[:, et : et + 1],
            0.0,
            op0=mybir.AluOpType.add,
            op1=mybir.AluOpType.max,
        )
        emb_sb[(2, et)] = t
    prim_chain(2)
    if STOP == 3:
        debug_out(
            lambda o: nc.vector.tensor_copy(
                o[:, :64], u2_all[:BL, 0, :]
            )
        )
        return

    # ---------------- dynamic routing (batch-sharded) -------------------
    a_d = [dram.tile([NCLS, 192], F32, name=f"a_d{i}") for i in range(2)]
    ag_a = [
        dram.tile([N_CORES, NCLS, 192], F32, addr_space="Shared", name=f"ag_a{i}")
        for i in range(2)
    ]
    c_dram = [dram.tile([192, NCLS], F32, name=f"c_dram{i}") for i in range(2)]
    b_cur = None  # [2,192] logits tile

    v_both = None
    for it in range(3):
        rnd = it - 1
        if it == 0:
            mset = w2_sb  # uniform c folded into evac scale 1/192
        else:
            # softmax(b_cur) over routes -> c_sm [2,192]
            mx = work.tile([NCLS, 1], F32, tag="smx", name="smx")
            nc.vector.tensor_reduce(
                mx[:], b_cur[:], axis=mybir.AxisListType.X, op=mybir.AluOpType.max
            )
            mxn = work.tile([NCLS, 1], F32, tag="smxn", name="smxn")
            nc.vector.tensor_scalar_mul(mxn[:], mx[:], -1.0)
            ex = work.tile([NCLS, 192], F32, tag="sex", name="sex")
            nc.scalar.activation(
                ex[:], b_cur[:], mybir.ActivationFunctionType.Exp, bias=mxn[:]
            )
            sm = work.tile([NCLS, 1], F32, tag="ssm", name="ssm")
            nc.vector.tensor_reduce(
                sm[:], ex[:], axis=mybir.AxisListType.X, op=mybir.AluOpType.add
            )
            rcp = work.tile([NCLS, 1], F32, tag="srcp", name="srcp")
            nc.vector.reciprocal(rcp[:], sm[:])
            c_sm = work.tile([NCLS, 192], F32, tag="scs", name="scs")
            nc.vector.tensor_scalar(
                c_sm[:], ex[:], rcp[:], None, op0=mybir.AluOpType.mult
            )
            # c [2,192] -> DRAM [192,2] -> broadcast-read c_exp [128, RT, 2]
            nc.sync.dma_start(c_dram[rnd][:].rearrange("r c -> c r"), c_sm[:])
            c_exp = work.tile([128, RT, NCLS], F32, tag="cexp", name="cexp")
            for t_ in range(RT):
                eng = (nc.sync, nc.scalar)[t_ % 2]
                eng.dma_start(
                    c_exp[:, t_, :],
                    c_dram[rnd][16 * t_ : 16 * (t_ + 1), :]
                    .broadcast_to([16, NCLS, 8])
                    .rearrange("j c r -> j r c"),
                )
            # mset[t] = w2[t] * c  (both classes in one op; vector/gpsimd)
            mset = []
            for t_ in range(RT):
                msc = work.tile([128, 128], BF, tag="msc", bufs=12, name="msc")
                nc.vector.tensor_tensor(
                    msc[:].rearrange("p (c o) -> p c o", c=NCLS),
                    w2_sb[t_][:].rearrange("p (c o) -> p c o", c=NCLS),
                    c_exp[:, t_, :].broadcast_to([128, NCLS, OC]),
                    op=mybir.AluOpType.mult,
                )
                mset.append(msc)

        # s per class
        if it > 0:
            s_ps = [
                ps1.tile([OC, BL], F32, tag=f"sps{c}", bufs=1, name=f"sps{c}")
                for c in range(NCLS)
            ]
        v_both = []
        for c in range(NCLS):
            t_start = 8 if it == 0 else 0
            for t_ in range(t_start, RT):
                lh = mset[t_][:, c * OC : (c + 1) * OC]
                nc.tensor.matmul(
                    s_ps[c][:],
                    lh,
                    u2_all[:, t_, :],
                    start=(it > 0 and t_ == 0),
                    stop=(t_ == RT - 1),
                )
            s_sb = work.tile([OC, BL], F32, tag=f"ssb{c}", name=f"ssb{c}")
            nc.vector.tensor_scalar_mul(
                s_sb[:], s_ps[c][:], (1.0 / 192.0) if it == 0 else 1.0
            )
            # elementwise digit squash v = s*|s|/(1+s^2) on [OC, BL]
            sq = work.tile([OC, BL], F32, tag=f"dsq{c}", name=f"dsq{c}")
            nc.vector.tensor_mul(sq[:], s_sb[:], s_sb[:])
            den = work.tile([OC, BL], F32, tag=f"dd2{c}", name=f"dd2{c}")
            nc.vector.tensor_scalar_add(den[:], sq[:], 1.0)
            rec = work.tile([OC, BL], F32, tag=f"dd4{c}", name=f"dd4{c}")
            nc.vector.reciprocal_approx_fast(rec[:], den[:])
            ns = work.tile([OC, BL], F32, tag=f"dn{c}", name=f"dn{c}")
            nc.vector.tensor_scalar_mul(ns[:], s_sb[:], -1.0)
            sab = work.tile([OC, BL], F32, tag=f"dd1{c}", name=f"dd1{c}")
            nc.vector.tensor_tensor(
                sab[:], s_sb[:], ns[:], op=mybir.AluOpType.max
            )
            num = work.tile([OC, BL], F32, tag=f"dd3{c}", name=f"dd3{c}")
            nc.vector.tensor_mul(num[:], sab[:], s_sb[:])
            vv = work.tile(
                [OC, BL],
                BF if it < 2 else F32,
                tag=f"vb{c}{'f' if it == 2 else ''}",
                name=f"vb{c}",
                bufs=2,
            )
            nc.vector.tensor_mul(vv[:], num[:], rec[:])
            v_both.append(vv)

        if it < 2:
            # agreement: abar[r,c] = sum_b sum_i u2 * (W3_c^T @ v_c)
            for c in range(NCLS):
                pc_all = ps1.tile(
                    [128, RT * BL], F32, tag="pca", bufs=1, name="pca"
                )
                for t_ in range(RT):
                    nc.tensor.matmul(
                        pc_all[:, t_ * BL : (t_ + 1) * BL],
                        w3_sb[c][:, t_ * 128 : (t_ + 1) * 128],
                        v_both[c][:],
                        start=True,
                        stop=True,
                    )
                prod = work.tile([128, RT * BL], BF, tag="prod", name="prod")
                nc.vector.tensor_mul(
                    prod[:], u2_all[:].rearrange("p t b -> p (t b)"), pc_all[:]
                )
                dcat = work.tile([128, RT], F32, tag=f"dcat{c}", name=f"dcat{c}")
                nc.vector.tensor_reduce(
                    dcat[:],
                    prod[:].rearrange("p (t b) -> p t b", t=RT),
                    axis=mybir.AxisListType.X,
                    op=mybir.AluOpType.add,
                )
                dtp = ps1.tile([RT, 128], F32, tag="pp", bufs=2, name="dtp")
                nc.tensor.transpose(dtp[:], dcat[:], ident_f[:])
                abar = work.tile([RT, 16], F32, tag=f"abar{c}", name=f"abar{c}")
                nc.vector.tensor_reduce(
                    abar[:],
                    dtp[:].rearrange("p (r i) -> p r i", i=8),
                    axis=mybir.AxisListType.X,
                    op=mybir.AluOpType.add,
                )
                nc.sync.dma_start(
                    a_d[it][c, :].rearrange("(t j) -> t j", t=RT), abar[:]
                )
            nc.gpsimd.collective_compute(
                "AllGather",
                mybir.AluOpType.bypass,
                replica_groups=rg,
                ins=[a_d[it][:]],
                outs=[ag_a[it][:]],
            )
            ld8 = work.tile(
                [NCLS, 192, N_CORES], F32, tag=f"agld{it}", name=f"agld{it}"
            )
            nc.sync.dma_start(ld8[:], ag_a[it][:].rearrange("r c n -> c n r"))
            asum = work.tile([NCLS, 192], F32, tag=f"asum{it}", name=f"asum{it}")
            nc.vector.tensor_reduce(
                asum[:], ld8[:], axis=mybir.AxisListType.X, op=mybir.AluOpType.add
            )
            b_new = work.tile([NCLS, 192], F32, tag=f"bcur{it}", name=f"bcur{it}")
            if it == 0:
                nc.vector.tensor_scalar_mul(b_new[:], asum[:], 1.0 / B)
            else:
                scaled = work.tile([NCLS, 192], F32, tag="arsc", name="arsc")
                nc.vector.tensor_scalar_mul(scaled[:], asum[:], 1.0 / B)
                nc.vector.tensor_add(b_new[:], b_cur[:], scaled[:])
            b_cur = b_new

    # final output: y[b, (c,o)] via PE transposes of v_c
    ob = work.tile([BL, 128], F32, tag="ob", bufs=2, name="ob")
    for c in range(NCLS):
        vt = ps1.tile([BL, OC], F32, tag="pp", bufs=2, name="vt")
        nc.tensor.transpose(vt[:], v_both[c][:], ident_f[:OC, :OC])
        nc.vector.tensor_copy(ob[:, c * OC : (c + 1) * OC], vt[:])
    nc.sync.dma_start(y[:, :], ob[:])
    ps1_cm.__exit__(None, None, None)


def _build_program():
    dt1 = BF if PHASE1_DT == "bf16" else F32
    nc = bacc.Bacc(num_devices=N_CORES)

    dlog_t = nc.declare_dram_parameter("dlog_t", [KC, B], dt1, isOutput=False)
    wp = nc.declare_dram_parameter("wp", [KC, E], dt1, isOutput=False)
    beta = nc.declare_dram_parameter("beta", [E], F32, isOutput=False)
    img_t = nc.declare_dram_parameter("img_t", [E, BL], BF, isOutput=False)
    capt_t = nc.declare_dram_parameter("capt_t", [E, BL], BF, isOutput=False)
    wm2 = nc.declare_dram_parameter("wm2", [3, E, 512], BF, isOutput=False)
    bias3 = nc.declare_dram_parameter("bias3", [3, 512], BF, isOutput=False)
    w2 = nc.declare_dram_parameter("w2", [RI, 128], BF, isOutput=False)
    w3 = nc.declare_dram_parameter("w3", [128, RI], BF, isOutput=False)
    y = nc.declare_dram_parameter("y", [BL, 128], F32, isOutput=True)
    io = (dlog_t, wp, beta, img_t, capt_t, wm2, bias3, w2, w3, y)

    with tile.TileContext(nc) as tc:
        with (
            tc.tile_pool(name="const", bufs=1) as const,
            tc.tile_pool(name="loads", bufs=3) as loads,
            tc.tile_pool(name="work", bufs=2) as work,
            tc.tile_pool(name="dram", bufs=1, space="DRAM") as dram,
        ):
            _emit(nc, tc, const, loads, work, dram, io)

    nc.compile()
    return nc


def _host_prep(inputs):
    """Numpy-side sharding/layout prep. Returns per-core input maps."""
    img_emb = np.asarray(inputs["img_emb"], dtype=np.float32)
    capt_emb = np.asarray(inputs["capt_emb"], dtype=np.float32)
    dct = np.asarray(inputs["DCT_features"], dtype=np.float32).reshape(B, K)
    w_emb = np.asarray(inputs["W_emb"], dtype=np.float32)
    b_emb = np.asarray(inputs["b_emb"], dtype=np.float32)
    w_digit = np.asarray(inputs["W_digit"], dtype=np.float32)

    dlog = np.log(np.abs(dct) + 1e-12)
    mu = float(dlog.mean(dtype=np.float64))
    sigma = float(dlog.std(ddof=1, dtype=np.float64))
    s_w = w_emb.sum(axis=1, dtype=np.float64)
    beta = (b_emb - (mu / sigma) * s_w).astype(np.float32)

    np_dt1 = _BF16 if PHASE1_DT == "bf16" else np.float32
    dlog_T = np.ascontiguousarray(dlog.T).astype(np_dt1)  # [K, B]
    wp = np.ascontiguousarray(w_emb.T / sigma).astype(np_dt1)  # [K, E]

    wm2 = np.stack(
        [
            np.ascontiguousarray(
                np.asarray(inputs[f"W_{m}"], dtype=np.float32).transpose(2, 1, 0)
            ).reshape(E, 512)
            for m in ("img", "capt", "dct")
        ]
    ).astype(_BF16)  # [3, E, 512]
    bias3 = np.stack(
        [
            np.ascontiguousarray(
                np.asarray(inputs[f"b_{m}"], dtype=np.float32).T
            ).reshape(512)
            for m in ("img", "capt", "dct")
        ]
    ).astype(_BF16)  # [3, 512]
    w2 = (
        np.ascontiguousarray(w_digit.transpose(0, 3, 1, 2))
        .reshape(RI, 128)
        .astype(_BF16)
    )
    w3 = np.concatenate(
        [
            np.ascontiguousarray(w_digit[:, c].transpose(1, 0, 2)).reshape(OC, RI)
            for c in range(NCLS)
        ]
    ).astype(_BF16)  # [128, RI]
    img_T = np.ascontiguousarray(img_emb.T).astype(_BF16)  # [E, B]
    capt_T = np.ascontiguousarray(capt_emb.T).astype(_BF16)

    in_maps = []
    for c in range(N_CORES):
        in_maps.append(
            {
                "dlog_t": np.ascontiguousarray(dlog_T[c * KC : (c + 1) * KC]),
                "wp": np.ascontiguousarray(wp[c * KC : (c + 1) * KC]),
                "beta": beta,
                "img_t": np.ascontiguousarray(img_T[:, c * BL : (c + 1) * BL]),
                "capt_t": np.ascontiguousarray(capt_T[:, c * BL : (c + 1) * BL]),
                "wm2": wm2,
                "bias3": bias3,
                "w2": w2,
                "w3": w3,
            }
        )
    return in_maps


def kernel(**inputs) -> np.ndarray:
    if "nc" not in _CACHE:
        _CACHE["nc"] = _build_program()
    nc = _CACHE["nc"]
    in_maps = _host_prep(inputs)
    trace = bool(int(os.environ.get("CAPS_TRACE", "0")))
    res = run_bass_kernel_spmd(nc, in_maps, list(range(N_CORES)), trace=trace)
    _CACHE["last_result"] = res
    out = np.concatenate([r["y"] for r in res.results], axis=0).reshape(
        B, NCLS, OC
    )
    return np.ascontiguousarray(out)[:, :, :, None]


# revision 15
# speedup vs baseline: 1.1121x; 1.1121x over previous
"""CapsNet Trainium2 kernel: 8-core SPMD Bass/Tile implementation.

Strategy (v3):
  Phase 1 (contraction-parallel): the dominant op is
     dct_emb = relu(norm(log|DCT|) @ W_emb.T + b_emb),  [512,102400]x[102400,768]
  Each core owns a 12800-wide slice of the 102400 contraction dim
  (~128us of bf16 PE time — the roofline). log|x| is precomputed on
  host (needed there for the global mean/std anyway) and the affine
  normalization folds into the matmul epilogue. Phase-1 streaming loads
  own the Sync DMA queue; constant loads ride other engine queues.

  Phase 2 (batch-parallel): the per-core partial G^T[768,512] products
  are combined with a bf16 ReduceScatter over the batch dim (DRAM
  layout [8, 768, 64] so rank r receives exactly its 64 batch columns).
  Each core runs primary caps + dynamic routing for its own 64 rows;
  the two batch-mean agreement reductions are tiny [2,192] AllReduces.
  The host concatenates the per-core [64,128] outputs.

  Dynamic routing never materializes u_hat[B,192,2,64]:
     s_c   = (W2 * c)^T @ u           (contraction over (r,i)=1536)
     P_c   = W3_c^T @ v_c             (back-projected v)
     a_rc  = sum_b sum_i u * P_c      (agreement, mul+reduce)
  The elementwise digit squash uses s*|s|/(1+s^2) (the 1e-7 eps only
  matters at |s|~3e-4 where the output is ~1e-11 — far below
  tolerance), so routing needs no scalar-engine table switches.
"""

import os
import sys

import numpy as np

if "/opt/trn_rl_repo" not in sys.path:
    sys.path.insert(0, "/opt/trn_rl_repo")

import concourse.bass as bass  # noqa: E402
import concourse.mybir as mybir  # noqa: E402
import concourse.tile as tile  # noqa: E402
from concourse import bacc  # noqa: E402
from concourse.bass_utils import run_bass_kernel_spmd  # noqa: E402
from concourse.masks import make_identity  # noqa: E402

try:
    import ml_dtypes  # noqa: E402

    _BF16 = ml_dtypes.bfloat16
except Exception:  # pragma: no cover
    _BF16 = None

N_CORES = 8
B = 512  # global batch
BL = B // N_CORES  # 64 local batch rows per core
K, KC = 102400, 12800  # contraction dim, per-core slice
E = 768  # embedding
ET = E // 128  # e chunks
KT = KC // 128  # k tiles per core (100)
GROUP = 5  # k tiles per load DMA
RI = 1536  # (route, in_cap) flat = 192*8
RT = RI // 128  # 12 tiles
NCLS = 2
OC = 64  # out caps channels
F32 = mybir.dt.float32
BF = mybir.dt.bfloat16

PHASE1_DT = os.environ.get("CAPS_P1_DT", "bf16")
RS_DT = os.environ.get("CAPS_RS_DT", "bf16")
# bisection: 1=phase1 only, 2=+RS, 3=+prim/squash/u2, 5=full
STOP = int(os.environ.get("CAPS_STOP", "5"))

_CACHE = {}


def _emit(nc, tc, const, loads, work, dram, io):
    dt1 = BF if PHASE1_DT == "bf16" else F32
    dt_rs = BF if RS_DT == "bf16" else F32
    rg = [list(range(N_CORES))]
    dlog_t, wp, beta, img_t, capt_t, wm2, bias3, w2, w3, y = io

    def debug_out(fill=None):
        out_sb = work.tile([BL, 128], F32, tag="outsb", name="outsb")
        nc.vector.memset(out_sb[:], 0.0)
        if fill is not None:
            fill(out_sb)
        nc.sync.dma_start(y[:, :], out_sb[:])

    # ---------------- phase 1: big matmul (loads on Sync queue) --------
    psA_cm = tc.tile_pool(name="psA", bufs=1, space="PSUM")
    psA = psA_cm.__enter__()
    g_ps = [psA.tile([128, B], F32, tag=f"g{ec}", name=f"g{ec}") for ec in range(ET)]
    n_loads = KT // GROUP

    def _emit_phase1():
        for li in range(n_loads):
            k0 = li * GROUP * 128
            dlog = loads.tile([128, GROUP, B], dt1, tag="dlog")
            nc.sync.dma_start(
                dlog[:],
                dlog_t[k0 : k0 + GROUP * 128, :].rearrange(
                    "(s p) b -> p s b", p=128
                ),
            )
            w_tile = loads.tile([128, GROUP, E], dt1, tag="w")
            nc.sync.dma_start(
                w_tile[:],
                wp[k0 : k0 + GROUP * 128, :].rearrange("(s p) e -> p s e", p=128),
            )
            for s in range(GROUP):
                kt = li * GROUP + s
                for ec in range(ET):
                    nc.tensor.matmul(
                        g_ps[ec][:],
                        w_tile[:, s, ec * 128 : (ec + 1) * 128],
                        dlog[:, s, :],
                        start=(kt == 0),
                        stop=(kt == KT - 1),
                    )

    # ---------------- constants (DMA on vector/scalar/tensor queues) ----
    eps_sq = const.tile([128, 1], F32)
    nc.vector.memset(eps_sq[:], 1e-7)
    ident_bf = const.tile([128, 128], BF)
    make_identity(nc, ident_bf[:])
    ident_f = const.tile([128, 128], F32)
    make_identity(nc, ident_f[:])
    beta_sb = const.tile([128, ET], F32)
    nc.gpsimd.dma_start(beta_sb[:], beta[:].rearrange("(t p) -> p t", p=128))
    emb_sb = {}  # (m, et) -> [128, BL] bf16 tile (lhsT source for prim)
    for m, src in ((0, img_t), (1, capt_t)):
        for et in range(ET):
            t = const.tile([128, BL], BF, tag=f"emb{m}_{et}", name=f"emb{m}_{et}")
            nc.gpsimd.dma_start(t[:], src[et * 128 : (et + 1) * 128, :])
            emb_sb[(m, et)] = t
    wm2_sb = {}
    for m in range(3):
        for et in range(ET):
            t = const.tile([128, 512], BF, tag=f"wm2_{m}_{et}", name=f"wm2_{m}_{et}")
            nc.scalar.dma_start(t[:], wm2[m, et * 128 : (et + 1) * 128, :])
            wm2_sb[(m, et)] = t
    bias_sb = []
    for m in range(3):
        t = const.tile([1, 512], BF, tag=f"bias{m}", name=f"bias{m}")
        nc.scalar.dma_start(t[:], bias3[m : m + 1, :])
        bias_sb.append(t)
    ones1 = const.tile([1, BL], BF)
    nc.vector.memset(ones1[:], 1.0)
    w2_sb = []
    for t_ in range(RT):
        t = const.tile([128, 128], BF, tag=f"w2_{t_}", name=f"w2_{t_}")
        nc.gpsimd.dma_start(t[:], w2[t_ * 128 : (t_ + 1) * 128, :])
        w2_sb.append(t)
    w3_sb = []
    for c in range(NCLS):
        t = const.tile([OC, RI], BF, tag=f"w3_{c}", name=f"w3_{c}")
        nc.gpsimd.dma_start(t[:], w3[c * OC : (c + 1) * OC, :])
        w3_sb.append(t)

    _emit_phase1()

    # evacuate PSUM -> SBUF -> cc_in[8, 768, 64]; ReduceScatter over batch
    cc_in = dram.tile([N_CORES, E, BL], dt_rs)
    for ec in range(ET):
        g_sb = work.tile([128, B], dt_rs, tag="gsb", bufs=2)
        if ec % 2 == 0:
            nc.vector.tensor_copy(g_sb[:], g_ps[ec][:])
        else:
            nc.scalar.copy(g_sb[:], g_ps[ec][:])
        nc.sync.dma_start(
            cc_in[:, ec * 128 : (ec + 1) * 128, :].rearrange("r p b -> p r b"),
            g_sb[:].rearrange("p (r b) -> p r b", r=N_CORES),
        )
    psA_cm.__exit__(None, None, None)
    ps1_cm = tc.tile_pool(name="psB", bufs=1, space="PSUM")
    ps1 = ps1_cm.__enter__()
    if STOP == 1:
        debug_out()
        return

    rs_out = dram.tile([E, BL], dt_rs)
    nc.gpsimd.collective_compute(
        "ReduceScatter",
        mybir.AluOpType.add,
        replica_groups=rg,
        ins=[cc_in[:]],
        outs=[rs_out[:]],
    )
    if STOP == 2:
        debug_out(lambda o: nc.sync.dma_start(o[:, :64], rs_out[:BL, :]))
        return

    # ---------------- primary caps + squash + transpose to u2 ----------
    u2_all = const.tile([128, RT, BL], BF)  # [(r,i)-tile, t, b]

    def prim_chain(m):
        pm = ps1.tile([BL, 512], F32, tag="pp", bufs=2, name="pm")
        for et in range(ET):
            nc.tensor.matmul(
                pm[:],
                emb_sb[(m, et)][:],
                wm2_sb[(m, et)][:],
                start=(et == 0),
                stop=False,
            )
        nc.tensor.matmul(pm[:], ones1[:], bias_sb[m][:], start=False, stop=True)
        upre_c = work.tile([BL, 512], F32, tag="upre", bufs=3, name="upre")
        nc.vector.tensor_copy(upre_c[:], pm[:])
        # squash over i (groups of 8 in the free dim), 64 routes here
        sq8 = work.tile([BL, 512], F32, tag="sq8", bufs=2)
        nc.vector.tensor_mul(sq8[:], upre_c[:], upre_c[:])
        usq = work.tile([BL, 64], F32, tag="usq")
        nc.vector.tensor_reduce(
            usq[:],
            sq8[:].rearrange("p (r i) -> p r i", i=8),
            axis=mybir.AxisListType.X,
            op=mybir.AluOpType.add,
        )
        t1 = work.tile([BL, 64], F32, tag="fa")
        nc.scalar.activation(
            t1[:], usq[:], mybir.ActivationFunctionType.Sqrt, bias=eps_sq[:BL, :]
        )
        t2 = work.tile([BL, 64], F32, tag="fb")
        nc.vector.tensor_scalar_add(t2[:], usq[:], 1.0)
        t3 = work.tile([BL, 64], F32, tag="fc")
        nc.vector.tensor_mul(t3[:], t1[:], t2[:])
        t4 = work.tile([BL, 64], F32, tag="fd")
        nc.vector.reciprocal_approx_fast(t4[:], t3[:])
        t5 = work.tile([BL, 64], F32, tag="fe")
        nc.vector.tensor_mul(t5[:], t4[:], usq[:])
        usquash = work.tile([BL, 512], BF, tag="usquash", bufs=2)
        nc.vector.tensor_tensor(
            usquash[:].rearrange("p (r i) -> p r i", i=8),
            upre_c[:].rearrange("p (r i) -> p r i", i=8),
            t5[:].broadcast_to([BL, 64, 8]),
            op=mybir.AluOpType.mult,
        )
        for j in range(4):
            tp = ps1.tile([128, BL], BF, tag="pp", bufs=2, name="tp")
            nc.tensor.transpose(
                tp[:], usquash[:, j * 128 : (j + 1) * 128], ident_bf[:BL, :BL]
            )
            nc.vector.tensor_copy(u2_all[:, 4 * m + j, :], tp[:])

    # img/capt chains execute on the PE during the ReduceScatter
    prim_chain(0)
    prim_chain(1)

    # iter-0 s-matmul partial accumulation over img/capt tiles (RS shadow)
    s_ps = [
        ps1.tile([OC, BL], F32, tag=f"sps{c}", bufs=1, name=f"sps{c}")
        for c in range(NCLS)
    ]
    for c in range(NCLS):
        for t_ in range(8):
            nc.tensor.matmul(
                s_ps[c][:],
                w2_sb[t_][:, c * OC : (c + 1) * OC],
                u2_all[:, t_, :],
                start=(t_ == 0),
                stop=False,
            )

    # dct embedding tiles: relu(rs_out + beta) -> bf16 (vector, no tables)
    gp = work.tile([128, ET, BL], dt_rs, tag="gp")
    nc.sync.dma_start(gp[:], rs_out[:].rearrange("(t p) b -> p t b", p=128))
    for et in range(ET):
        t = const.tile([128, BL], BF, tag=f"emb2_{et}", name=f"emb2_{et}")
        nc.vector.tensor_scalar(
            t[:],
            gp[:, et, :],
            beta_sb[:, et : et + 1],
            0.0,
            op0=mybir.AluOpType.add,
            op1=mybir.AluOpType.max,
        )
        emb_sb[(2, et)] = t
    prim_chain(2)
    if STOP == 3:
        debug_out(
            lambda o: nc.vector.tensor_copy(
                o[:, :64], u2_all[:BL, 0, :]
            )
        )
        return

    # ---------------- dynamic routing (batch-sharded) -------------------
    a_d = [dram.tile([NCLS, 192], F32, name=f"a_d{i}") for i in range(2)]
    ar_a = [
        dram.tile([NCLS, 192], F32, addr_space="Shared", name=f"ar_a{i}")
        for i in range(2)
    ]
    c_dram = [dram.tile([192, NCLS], F32, name=f"c_dram{i}") for i in range(2)]
    b_cur = None  # [2,192] logits tile

    v_both = None
    for it in range(3):
        rnd = it - 1
        if it == 0:
            mset = w2_sb  # uniform c folded into evac scale 1/192
        else:
            # softmax(b_cur) over routes -> c_sm [2,192]
            mx = work.tile([NCLS, 1], F32, tag="smx", name="smx")
            nc.vector.tensor_reduce(
                mx[:], b_cur[:], axis=mybir.AxisListType.X, op=mybir.AluOpType.max
            )
            mxn = work.tile([NCLS, 1], F32, tag="smxn", name="smxn")
            nc.vector.tensor_scalar_mul(mxn[:], mx[:], -1.0)
            ex = work.tile([NCLS, 192], F32, tag="sex", name="sex")
            nc.scalar.activation(
                ex[:], b_cur[:], mybir.ActivationFunctionType.Exp, bias=mxn[:]
            )
            sm = work.tile([NCLS, 1], F32, tag="ssm", name="ssm")
            nc.vector.tensor_reduce(
                sm[:], ex[:], axis=mybir.AxisListType.X, op=mybir.AluOpType.add
            )
            rcp = work.tile([NCLS, 1], F32, tag="srcp", name="srcp")
            nc.vector.reciprocal(rcp[:], sm[:])
            c_sm = work.tile([NCLS, 192], F32, tag="scs", name="scs")
            nc.vector.tensor_scalar(
                c_sm[:], ex[:], rcp[:], None, op0=mybir.AluOpType.mult
            )
            # c [2,192] -> DRAM [192,2] -> broadcast-read c_exp [128, RT, 2]
            nc.sync.dma_start(c_dram[rnd][:].rearrange("r c -> c r"), c_sm[:])
            c_exp = work.tile([128, RT, NCLS], F32, tag="cexp", name="cexp")
            for t_ in range(RT):
                eng = (nc.sync, nc.scalar)[t_ % 2]
                eng.dma_start(
                    c_exp[:, t_, :],
                    c_dram[rnd][16 * t_ : 16 * (t_ + 1), :]
                    .broadcast_to([16, NCLS, 8])
                    .rearrange("j c r -> j r c"),
                )
            # mset[t] = w2[t] * c  (both classes in one op; vector/gpsimd)
            mset = []
            for t_ in range(RT):
                msc = work.tile([128, 128], BF, tag="msc", bufs=12, name="msc")
                nc.vector.tensor_tensor(
                    msc[:].rearrange("p (c o) -> p c o", c=NCLS),
                    w2_sb[t_][:].rearrange("p (c o) -> p c o", c=NCLS),
                    c_exp[:, t_, :].broadcast_to([128, NCLS, OC]),
                    op=mybir.AluOpType.mult,
                )
                mset.append(msc)

        # s per class
        if it > 0:
            s_ps = [
                ps1.tile([OC, BL], F32, tag=f"sps{c}", bufs=1, name=f"sps{c}")
                for c in range(NCLS)
            ]
        v_both = []
        for c in range(NCLS):
            t_start = 8 if it == 0 else 0
            for t_ in range(t_start, RT):
                lh = mset[t_][:, c * OC : (c + 1) * OC]
                nc.tensor.matmul(
                    s_ps[c][:],
                    lh,
                    u2_all[:, t_, :],
                    start=(it > 0 and t_ == 0),
                    stop=(t_ == RT - 1),
                )
            s_sb = work.tile([OC, BL], F32, tag=f"ssb{c}", name=f"ssb{c}")
            nc.vector.tensor_scalar_mul(
                s_sb[:], s_ps[c][:], (1.0 / 192.0) if it == 0 else 1.0
            )
            # elementwise digit squash v = s*|s|/(1+s^2) on [OC, BL]
            sq = work.tile([OC, BL], F32, tag=f"dsq{c}", name=f"dsq{c}")
            nc.vector.tensor_mul(sq[:], s_sb[:], s_sb[:])
            den = work.tile([OC, BL], F32, tag=f"dd2{c}", name=f"dd2{c}")
            nc.vector.tensor_scalar_add(den[:], sq[:], 1.0)
            rec = work.tile([OC, BL], F32, tag=f"dd4{c}", name=f"dd4{c}")
            nc.vector.reciprocal_approx_fast(rec[:], den[:])
            ns = work.tile([OC, BL], F32, tag=f"dn{c}", name=f"dn{c}")
            nc.vector.tensor_scalar_mul(ns[:], s_sb[:], -1.0)
            sab = work.tile([OC, BL], F32, tag=f"dd1{c}", name=f"dd1{c}")
            nc.vector.tensor_tensor(
                sab[:], s_sb[:], ns[:], op=mybir.AluOpType.max
            )
            num = work.tile([OC, BL], F32, tag=f"dd3{c}", name=f"dd3{c}")
            nc.vector.tensor_mul(num[:], sab[:], s_sb[:])
            vv = work.tile(
                [OC, BL],
                BF if it < 2 else F32,
                tag=f"vb{c}{'f' if it == 2 else ''}",
                name=f"vb{c}",
                bufs=2,
            )
            nc.vector.tensor_mul(vv[:], num[:], rec[:])
            v_both.append(vv)

        if it < 2:
            # agreement: abar[r,c] = sum_b sum_i u2 * (W3_c^T @ v_c)
            for c in range(NCLS):
                pc_all = ps1.tile(
                    [128, RT * BL], F32, tag="pca", bufs=1, name="pca"
                )
                for t_ in range(RT):
                    nc.tensor.matmul(
                        pc_all[:, t_ * BL : (t_ + 1) * BL],
                        w3_sb[c][:, t_ * 128 : (t_ + 1) * 128],
                        v_both[c][:],
                        start=True,
                        stop=True,
                    )
                prod = work.tile([128, RT * BL], BF, tag="prod", name="prod")
                nc.vector.tensor_mul(
                    prod[:], u2_all[:].rearrange("p t b -> p (t b)"), pc_all[:]
                )
                dcat = work.tile([128, RT], F32, tag=f"dcat{c}", name=f"dcat{c}")
                nc.vector.tensor_reduce(
                    dcat[:],
                    prod[:].rearrange("p (t b) -> p t b", t=RT),
                    axis=mybir.AxisListType.X,
                    op=mybir.AluOpType.add,
                )
                dtp = ps1.tile([RT, 128], F32, tag="pp", bufs=2, name="dtp")
                nc.tensor.transpose(dtp[:], dcat[:], ident_f[:])
                abar = work.tile([RT, 16], F32, tag=f"abar{c}", name=f"abar{c}")
                nc.vector.tensor_reduce(
                    abar[:],
                    dtp[:].rearrange("p (r i) -> p r i", i=8),
                    axis=mybir.AxisListType.X,
                    op=mybir.AluOpType.add,
                )
                nc.sync.dma_start(
                    a_d[it][c, :].rearrange("(t j) -> t j", t=RT), abar[:]
                )
            nc.gpsimd.collective_compute(
                "AllReduce",
                mybir.AluOpType.add,
                replica_groups=rg,
                ins=[a_d[it][:]],
                outs=[ar_a[it][:]],
            )
            ld = work.tile([NCLS, 192], F32, tag=f"arld{it}", name=f"arld{it}")
            nc.sync.dma_start(ld[:], ar_a[it][:])
            b_new = work.tile([NCLS, 192], F32, tag=f"bcur{it}", name=f"bcur{it}")
            if it == 0:
                nc.vector.tensor_scalar_mul(b_new[:], ld[:], 1.0 / B)
            else:
                scaled = work.tile([NCLS, 192], F32, tag="arsc", name="arsc")
                nc.vector.tensor_scalar_mul(scaled[:], ld[:], 1.0 / B)
                nc.vector.tensor_add(b_new[:], b_cur[:], scaled[:])
            b_cur = b_new

    # final output: y[b, (c,o)] via PE transposes of v_c
    ob = work.tile([BL, 128], F32, tag="ob", bufs=2, name="ob")
    for c in range(NCLS):
        vt = ps1.tile([BL, OC], F32, tag="pp", bufs=2, name="vt")
        nc.tensor.transpose(vt[:], v_both[c][:], ident_f[:OC, :OC])
        nc.vector.tensor_copy(ob[:, c * OC : (c + 1) * OC], vt[:])
    nc.sync.dma_start(y[:, :], ob[:])
    ps1_cm.__exit__(None, None, None)


def _build_program():
    dt1 = BF if PHASE1_DT == "bf16" else F32
    nc = bacc.Bacc(num_devices=N_CORES)

    dlog_t = nc.declare_dram_parameter("dlog_t", [KC, B], dt1, isOutput=False)
    wp = nc.declare_dram_parameter("wp", [KC, E], dt1, isOutput=False)
    beta = nc.declare_dram_parameter("beta", [E], F32, isOutput=False)
    img_t = nc.declare_dram_parameter("img_t", [E, BL], BF, isOutput=False)
    capt_t = nc.declare_dram_parameter("capt_t", [E, BL], BF, isOutput=False)
    wm2 = nc.declare_dram_parameter("wm2", [3, E, 512], BF, isOutput=False)
    bias3 = nc.declare_dram_parameter("bias3", [3, 512], BF, isOutput=False)
    w2 = nc.declare_dram_parameter("w2", [RI, 128], BF, isOutput=False)
    w3 = nc.declare_dram_parameter("w3", [128, RI], BF, isOutput=False)
    y = nc.declare_dram_parameter("y", [BL, 128], F32, isOutput=True)
    io = (dlog_t, wp, beta, img_t, capt_t, wm2, bias3, w2, w3, y)

    with tile.TileContext(nc) as tc:
        with (
            tc.tile_pool(name="const", bufs=1) as const,
            tc.tile_pool(name="loads", bufs=3) as loads,
            tc.tile_pool(name="work", bufs=2) as work,
            tc.tile_pool(name="dram", bufs=1, space="DRAM") as dram,
        ):
            _emit(nc, tc, const, loads, work, dram, io)

    nc.compile()
    return nc


def _host_prep(inputs):
    """Numpy-side sharding/layout prep. Returns per-core input maps."""
    img_emb = np.asarray(inputs["img_emb"], dtype=np.float32)
    capt_emb = np.asarray(inputs["capt_emb"], dtype=np.float32)
    dct = np.asarray(inputs["DCT_features"], dtype=np.float32).reshape(B, K)
    w_emb = np.asarray(inputs["W_emb"], dtype=np.float32)
    b_emb = np.asarray(inputs["b_emb"], dtype=np.float32)
    w_digit = np.asarray(inputs["W_digit"], dtype=np.float32)

    dlog = np.log(np.abs(dct) + 1e-12)
    mu = float(dlog.mean(dtype=np.float64))
    sigma = float(dlog.std(ddof=1, dtype=np.float64))
    s_w = w_emb.sum(axis=1, dtype=np.float64)
    beta = (b_emb - (mu / sigma) * s_w).astype(np.float32)

    np_dt1 = _BF16 if PHASE1_DT == "bf16" else np.float32
    dlog_T = np.ascontiguousarray(dlog.T).astype(np_dt1)  # [K, B]
    wp = np.ascontiguousarray(w_emb.T / sigma).astype(np_dt1)  # [K, E]

    wm2 = np.stack(
        [
            np.ascontiguousarray(
                np.asarray(inputs[f"W_{m}"], dtype=np.float32).transpose(2, 1, 0)
            ).reshape(E, 512)
            for m in ("img", "capt", "dct")
        ]
    ).astype(_BF16)  # [3, E, 512]
    bias3 = np.stack(
        [
            np.ascontiguousarray(
                np.asarray(inputs[f"b_{m}"], dtype=np.float32).T
            ).reshape(512)
            for m in ("img", "capt", "dct")
        ]
    ).astype(_BF16)  # [3, 512]
    w2 = (
        np.ascontiguousarray(w_digit.transpose(0, 3, 1, 2))
        .reshape(RI, 128)
        .astype(_BF16)
    )
    w3 = np.concatenate(
        [
            np.ascontiguousarray(w_digit[:, c].transpose(1, 0, 2)).reshape(OC, RI)
            for c in range(NCLS)
        ]
    ).astype(_BF16)  # [128, RI]
    img_T = np.ascontiguousarray(img_emb.T).astype(_BF16)  # [E, B]
    capt_T = np.ascontiguousarray(capt_emb.T).astype(_BF16)

    in_maps = []
    for c in range(N_CORES):
        in_maps.append(
            {
                "dlog_t": np.ascontiguousarray(dlog_T[c * KC : (c + 1) * KC]),
                "wp": np.ascontiguousarray(wp[c * KC : (c + 1) * KC]),
                "beta": beta,
                "img_t": np.ascontiguousarray(img_T[:, c * BL : (c + 1) * BL]),
                "capt_t": np.ascontiguousarray(capt_T[:, c * BL : (c + 1) * BL]),
                "wm2": wm2,
                "bias3": bias3,
                "w2": w2,
                "w3": w3,
            }
        )
    return in_maps


def kernel(**inputs) -> np.ndarray:
    if "nc" not in _CACHE:
        _CACHE["nc"] = _build_program()
    nc = _CACHE["nc"]
    in_maps = _host_prep(inputs)
    trace = bool(int(os.environ.get("CAPS_TRACE", "0")))
    res = run_bass_kernel_spmd(nc, in_maps, list(range(N_CORES)), trace=trace)
    _CACHE["last_result"] = res
    out = np.concatenate([r["y"] for r in res.results], axis=0).reshape(
        B, NCLS, OC
    )
    return np.ascontiguousarray(out)[:, :, :, None]


# revision 16
# speedup vs baseline: 1.1289x; 1.0151x over previous
"""CapsNet Trainium2 kernel: 8-core SPMD Bass/Tile implementation.

Strategy (v3):
  Phase 1 (contraction-parallel): the dominant op is
     dct_emb = relu(norm(log|DCT|) @ W_emb.T + b_emb),  [512,102400]x[102400,768]
  Each core owns a 12800-wide slice of the 102400 contraction dim
  (~128us of bf16 PE time — the roofline). log|x| is precomputed on
  host (needed there for the global mean/std anyway) and the affine
  normalization folds into the matmul epilogue. Phase-1 streaming loads
  own the Sync DMA queue; constant loads ride other engine queues.

  Phase 2 (batch-parallel): the per-core partial G^T[768,512] products
  are combined with a bf16 ReduceScatter over the batch dim (DRAM
  layout [8, 768, 64] so rank r receives exactly its 64 batch columns).
  Each core runs primary caps + dynamic routing for its own 64 rows;
  the two batch-mean agreement reductions are tiny [2,192] AllReduces.
  The host concatenates the per-core [64,128] outputs.

  Dynamic routing never materializes u_hat[B,192,2,64]:
     s_c   = (W2 * c)^T @ u           (contraction over (r,i)=1536)
     P_c   = W3_c^T @ v_c             (back-projected v)
     a_rc  = sum_b sum_i u * P_c      (agreement, mul+reduce)
  The elementwise digit squash uses s*|s|/(1+s^2) (the 1e-7 eps only
  matters at |s|~3e-4 where the output is ~1e-11 — far below
  tolerance), so routing needs no scalar-engine table switches.
"""

import os
import sys

import numpy as np

if "/opt/trn_rl_repo" not in sys.path:
    sys.path.insert(0, "/opt/trn_rl_repo")

import concourse.bass as bass  # noqa: E402
import concourse.mybir as mybir  # noqa: E402
import concourse.tile as tile  # noqa: E402
from concourse import bacc  # noqa: E402
from concourse.bass_utils import run_bass_kernel_spmd  # noqa: E402
from concourse.masks import make_identity  # noqa: E402

try:
    import ml_dtypes  # noqa: E402

    _BF16 = ml_dtypes.bfloat16
except Exception:  # pragma: no cover
    _BF16 = None

N_CORES = 8
B = 512  # global batch
BL = B // N_CORES  # 64 local batch rows per core
K, KC = 102400, 12800  # contraction dim, per-core slice
E = 768  # embedding
ET = E // 128  # e chunks
KT = KC // 128  # k tiles per core (100)
GROUP = 10  # k tiles per load DMA
RI = 1536  # (route, in_cap) flat = 192*8
RT = RI // 128  # 12 tiles
NCLS = 2
OC = 64  # out caps channels
F32 = mybir.dt.float32
BF = mybir.dt.bfloat16

PHASE1_DT = os.environ.get("CAPS_P1_DT", "bf16")
RS_DT = os.environ.get("CAPS_RS_DT", "bf16")
# bisection: 1=phase1 only, 2=+RS, 3=+prim/squash/u2, 5=full
STOP = int(os.environ.get("CAPS_STOP", "5"))

_CACHE = {}


def _emit(nc, tc, const, loads, work, dram, io):
    dt1 = BF if PHASE1_DT == "bf16" else F32
    dt_rs = BF if RS_DT == "bf16" else F32
    rg = [list(range(N_CORES))]
    dlog_t, wp, beta, img_t, capt_t, wm2, bias3, w2, w3, y = io

    def debug_out(fill=None):
        out_sb = work.tile([BL, 128], F32, tag="outsb", name="outsb")
        nc.vector.memset(out_sb[:], 0.0)
        if fill is not None:
            fill(out_sb)
        nc.sync.dma_start(y[:, :], out_sb[:])

    # ---------------- phase 1: big matmul (loads on Sync queue) --------
    psA_cm = tc.tile_pool(name="psA", bufs=1, space="PSUM")
    psA = psA_cm.__enter__()
    g_ps = [psA.tile([128, B], F32, tag=f"g{ec}", name=f"g{ec}") for ec in range(ET)]
    n_loads = KT // GROUP

    def _emit_phase1():
        for li in range(n_loads):
            k0 = li * GROUP * 128
            dlog = loads.tile([128, GROUP, B], dt1, tag="dlog")
            nc.sync.dma_start(
                dlog[:],
                dlog_t[k0 : k0 + GROUP * 128, :].rearrange(
                    "(s p) b -> p s b", p=128
                ),
            )
            w_tile = loads.tile([128, GROUP, E], dt1, tag="w")
            nc.sync.dma_start(
                w_tile[:],
                wp[k0 : k0 + GROUP * 128, :].rearrange("(s p) e -> p s e", p=128),
            )
            for s in range(GROUP):
                kt = li * GROUP + s
                for ec in range(ET):
                    nc.tensor.matmul(
                        g_ps[ec][:],
                        w_tile[:, s, ec * 128 : (ec + 1) * 128],
                        dlog[:, s, :],
                        start=(kt == 0),
                        stop=(kt == KT - 1),
                    )

    # ---------------- constants (DMA on vector/scalar/tensor queues) ----
    eps_sq = const.tile([128, 1], F32)
    nc.vector.memset(eps_sq[:], 1e-7)
    ident_bf = const.tile([128, 128], BF)
    make_identity(nc, ident_bf[:])
    ident_f = const.tile([128, 128], F32)
    make_identity(nc, ident_f[:])
    beta_sb = const.tile([128, ET], F32)
    emb_sb = {}  # (m, et) -> [128, BL] bf16 tile (lhsT source for prim)
    wm2_sb = {}
    bias_sb = []
    w2_sb = []
    w3_sb = []
    with tc.tile_wait_until(0.115):
        nc.gpsimd.dma_start(beta_sb[:], beta[:].rearrange("(t p) -> p t", p=128))
        for m, src in ((0, img_t), (1, capt_t)):
            for et in range(ET):
                t = const.tile(
                    [128, BL], BF, tag=f"emb{m}_{et}", name=f"emb{m}_{et}"
                )
                nc.gpsimd.dma_start(t[:], src[et * 128 : (et + 1) * 128, :])
                emb_sb[(m, et)] = t
        for m in range(3):
            for et in range(ET):
                t = const.tile(
                    [128, 512], BF, tag=f"wm2_{m}_{et}", name=f"wm2_{m}_{et}"
                )
                nc.scalar.dma_start(t[:], wm2[m, et * 128 : (et + 1) * 128, :])
                wm2_sb[(m, et)] = t
        for m in range(3):
            t = const.tile([1, 512], BF, tag=f"bias{m}", name=f"bias{m}")
            nc.scalar.dma_start(t[:], bias3[m : m + 1, :])
            bias_sb.append(t)
    ones1 = const.tile([1, BL], BF)
    nc.vector.memset(ones1[:], 1.0)
    with tc.tile_wait_until(0.15):
        for t_ in range(RT):
            t = const.tile([128, 128], BF, tag=f"w2_{t_}", name=f"w2_{t_}")
            nc.gpsimd.dma_start(t[:], w2[t_ * 128 : (t_ + 1) * 128, :])
            w2_sb.append(t)
        for c in range(NCLS):
            t = const.tile([OC, RI], BF, tag=f"w3_{c}", name=f"w3_{c}")
            nc.gpsimd.dma_start(t[:], w3[c * OC : (c + 1) * OC, :])
            w3_sb.append(t)

    _emit_phase1()

    # evacuate PSUM -> SBUF -> cc_in[8, 768, 64]; ReduceScatter over batch
    cc_in = dram.tile([N_CORES, E, BL], dt_rs)
    for ec in range(ET):
        g_sb = work.tile([128, B], dt_rs, tag="gsb", bufs=2)
        if ec % 2 == 0:
            nc.vector.tensor_copy(g_sb[:], g_ps[ec][:])
        else:
            nc.scalar.copy(g_sb[:], g_ps[ec][:])
        nc.sync.dma_start(
            cc_in[:, ec * 128 : (ec + 1) * 128, :].rearrange("r p b -> p r b"),
            g_sb[:].rearrange("p (r b) -> p r b", r=N_CORES),
        )
    psA_cm.__exit__(None, None, None)
    ps1_cm = tc.tile_pool(name="psB", bufs=1, space="PSUM")
    ps1 = ps1_cm.__enter__()
    if STOP == 1:
        debug_out()
        return

    rs_out = dram.tile([E, BL], dt_rs)
    nc.gpsimd.collective_compute(
        "ReduceScatter",
        mybir.AluOpType.add,
        replica_groups=rg,
        ins=[cc_in[:]],
        outs=[rs_out[:]],
    )
    if STOP == 2:
        debug_out(lambda o: nc.sync.dma_start(o[:, :64], rs_out[:BL, :]))
        return

    # ---------------- primary caps + squash + transpose to u2 ----------
    u2_all = const.tile([128, RT, BL], BF)  # [(r,i)-tile, t, b]

    def prim_chain(m):
        pm = ps1.tile([BL, 512], F32, tag="pp", bufs=2, name="pm")
        for et in range(ET):
            nc.tensor.matmul(
                pm[:],
                emb_sb[(m, et)][:],
                wm2_sb[(m, et)][:],
                start=(et == 0),
                stop=False,
            )
        nc.tensor.matmul(pm[:], ones1[:], bias_sb[m][:], start=False, stop=True)
        upre_c = work.tile([BL, 512], F32, tag="upre", bufs=3, name="upre")
        nc.vector.tensor_copy(upre_c[:], pm[:])
        # squash over i (groups of 8 in the free dim), 64 routes here
        sq8 = work.tile([BL, 512], F32, tag="sq8", bufs=2)
        nc.vector.tensor_mul(sq8[:], upre_c[:], upre_c[:])
        usq = work.tile([BL, 64], F32, tag="usq")
        nc.vector.tensor_reduce(
            usq[:],
            sq8[:].rearrange("p (r i) -> p r i", i=8),
            axis=mybir.AxisListType.X,
            op=mybir.AluOpType.add,
        )
        t1 = work.tile([BL, 64], F32, tag="fa")
        nc.scalar.activation(
            t1[:], usq[:], mybir.ActivationFunctionType.Sqrt, bias=eps_sq[:BL, :]
        )
        t2 = work.tile([BL, 64], F32, tag="fb")
        nc.vector.tensor_scalar_add(t2[:], usq[:], 1.0)
        t3 = work.tile([BL, 64], F32, tag="fc")
        nc.vector.tensor_mul(t3[:], t1[:], t2[:])
        t4 = work.tile([BL, 64], F32, tag="fd")
        nc.vector.reciprocal_approx_fast(t4[:], t3[:])
        t5 = work.tile([BL, 64], F32, tag="fe")
        nc.vector.tensor_mul(t5[:], t4[:], usq[:])
        usquash = work.tile([BL, 512], BF, tag="usquash", bufs=2)
        nc.vector.tensor_tensor(
            usquash[:].rearrange("p (r i) -> p r i", i=8),
            upre_c[:].rearrange("p (r i) -> p r i", i=8),
            t5[:].broadcast_to([BL, 64, 8]),
            op=mybir.AluOpType.mult,
        )
        for j in range(4):
            tp = ps1.tile([128, BL], BF, tag="pp", bufs=2, name="tp")
            nc.tensor.transpose(
                tp[:], usquash[:, j * 128 : (j + 1) * 128], ident_bf[:BL, :BL]
            )
            nc.vector.tensor_copy(u2_all[:, 4 * m + j, :], tp[:])

    # img/capt chains execute on the PE during the ReduceScatter
    prim_chain(0)
    prim_chain(1)

    # iter-0 s-matmul partial accumulation over img/capt tiles (RS shadow)
    s_ps = [
        ps1.tile([OC, BL], F32, tag=f"sps{c}", bufs=1, name=f"sps{c}")
        for c in range(NCLS)
    ]
    for c in range(NCLS):
        for t_ in range(8):
            nc.tensor.matmul(
                s_ps[c][:],
                w2_sb[t_][:, c * OC : (c + 1) * OC],
                u2_all[:, t_, :],
                start=(t_ == 0),
                stop=False,
            )

    # dct embedding tiles: relu(rs_out + beta) -> bf16 (vector, no tables)
    gp = work.tile([128, ET, BL], dt_rs, tag="gp")
    nc.sync.dma_start(gp[:], rs_out[:].rearrange("(t p) b -> p t b", p=128))
    for et in range(ET):
        t = const.tile([128, BL], BF, tag=f"emb2_{et}", name=f"emb2_{et}")
        nc.vector.tensor_scalar(
            t[:],
            gp[:, et, :],
            beta_sb[:, et : et + 1],
            0.0,
            op0=mybir.AluOpType.add,
            op1=mybir.AluOpType.max,
        )
        emb_sb[(2, et)] = t
    prim_chain(2)
    if STOP == 3:
        debug_out(
            lambda o: nc.vector.tensor_copy(
                o[:, :64], u2_all[:BL, 0, :]
            )
        )
        return

    # ---------------- dynamic routing (batch-sharded) -------------------
    a_d = [dram.tile([NCLS, 192], F32, name=f"a_d{i}") for i in range(2)]
    ar_a = [
        dram.tile([NCLS, 192], F32, addr_space="Shared", name=f"ar_a{i}")
        for i in range(2)
    ]
    c_dram = [dram.tile([192, NCLS], F32, name=f"c_dram{i}") for i in range(2)]
    b_cur = None  # [2,192] logits tile

    v_both = None
    for it in range(3):
        rnd = it - 1
        if it == 0:
            mset = w2_sb  # uniform c folded into evac scale 1/192
        else:
            # softmax(b_cur) over routes -> c_sm [2,192]
            mx = work.tile([NCLS, 1], F32, tag="smx", name="smx")
            nc.vector.tensor_reduce(
                mx[:], b_cur[:], axis=mybir.AxisListType.X, op=mybir.AluOpType.max
            )
            mxn = work.tile([NCLS, 1], F32, tag="smxn", name="smxn")
            nc.vector.tensor_scalar_mul(mxn[:], mx[:], -1.0)
            ex = work.tile([NCLS, 192], F32, tag="sex", name="sex")
            nc.scalar.activation(
                ex[:], b_cur[:], mybir.ActivationFunctionType.Exp, bias=mxn[:]
            )
            sm = work.tile([NCLS, 1], F32, tag="ssm", name="ssm")
            nc.vector.tensor_reduce(
                sm[:], ex[:], axis=mybir.AxisListType.X, op=mybir.AluOpType.add
            )
            rcp = work.tile([NCLS, 1], F32, tag="srcp", name="srcp")
            nc.vector.reciprocal(rcp[:], sm[:])
            c_sm = work.tile([NCLS, 192], F32, tag="scs", name="scs")
            nc.vector.tensor_scalar(
                c_sm[:], ex[:], rcp[:], None, op0=mybir.AluOpType.mult
            )
            # c [2,192] -> DRAM [192,2] -> broadcast-read c_exp [128, RT, 2]
            nc.sync.dma_start(c_dram[rnd][:].rearrange("r c -> c r"), c_sm[:])
            c_exp = work.tile([128, RT, NCLS], F32, tag="cexp", name="cexp")
            for t_ in range(RT):
                eng = (nc.sync, nc.scalar)[t_ % 2]
                eng.dma_start(
                    c_exp[:, t_, :],
                    c_dram[rnd][16 * t_ : 16 * (t_ + 1), :]
                    .broadcast_to([16, NCLS, 8])
                    .rearrange("j c r -> j r c"),
                )
            # mset[t] = w2[t] * c  (both classes in one op; vector/gpsimd)
            mset = []
            for t_ in range(RT):
                msc = work.tile([128, 128], BF, tag="msc", bufs=12, name="msc")
                nc.vector.tensor_tensor(
                    msc[:].rearrange("p (c o) -> p c o", c=NCLS),
                    w2_sb[t_][:].rearrange("p (c o) -> p c o", c=NCLS),
                    c_exp[:, t_, :].broadcast_to([128, NCLS, OC]),
                    op=mybir.AluOpType.mult,
                )
                mset.append(msc)

        # s per class
        if it > 0:
            s_ps = [
                ps1.tile([OC, BL], F32, tag=f"sps{c}", bufs=1, name=f"sps{c}")
                for c in range(NCLS)
            ]
        v_both = []
        for c in range(NCLS):
            t_start = 8 if it == 0 else 0
            for t_ in range(t_start, RT):
                lh = mset[t_][:, c * OC : (c + 1) * OC]
                nc.tensor.matmul(
                    s_ps[c][:],
                    lh,
                    u2_all[:, t_, :],
                    start=(it > 0 and t_ == 0),
                    stop=(t_ == RT - 1),
                )
            s_sb = work.tile([OC, BL], F32, tag=f"ssb{c}", name=f"ssb{c}")
            nc.vector.tensor_scalar_mul(
                s_sb[:], s_ps[c][:], (1.0 / 192.0) if it == 0 else 1.0
            )
            # elementwise digit squash v = s*|s|/(1+s^2) on [OC, BL]
            sq = work.tile([OC, BL], F32, tag=f"dsq{c}", name=f"dsq{c}")
            nc.vector.tensor_mul(sq[:], s_sb[:], s_sb[:])
            den = work.tile([OC, BL], F32, tag=f"dd2{c}", name=f"dd2{c}")
            nc.vector.tensor_scalar_add(den[:], sq[:], 1.0)
            rec = work.tile([OC, BL], F32, tag=f"dd4{c}", name=f"dd4{c}")
            nc.vector.reciprocal_approx_fast(rec[:], den[:])
            ns = work.tile([OC, BL], F32, tag=f"dn{c}", name=f"dn{c}")
            nc.vector.tensor_scalar_mul(ns[:], s_sb[:], -1.0)
            sab = work.tile([OC, BL], F32, tag=f"dd1{c}", name=f"dd1{c}")
            nc.vector.tensor_tensor(
                sab[:], s_sb[:], ns[:], op=mybir.AluOpType.max
            )
            num = work.tile([OC, BL], F32, tag=f"dd3{c}", name=f"dd3{c}")
            nc.vector.tensor_mul(num[:], sab[:], s_sb[:])
            vv = work.tile(
                [OC, BL],
                BF if it < 2 else F32,
                tag=f"vb{c}{'f' if it == 2 else ''}",
                name=f"vb{c}",
                bufs=2,
            )
            nc.vector.tensor_mul(vv[:], num[:], rec[:])
            v_both.append(vv)

        if it < 2:
            # agreement: abar[r,c] = sum_b sum_i u2 * (W3_c^T @ v_c)
            for c in range(NCLS):
                pc_all = ps1.tile(
                    [128, RT * BL], F32, tag="pca", bufs=1, name="pca"
                )
                for t_ in range(RT):
                    nc.tensor.matmul(
                        pc_all[:, t_ * BL : (t_ + 1) * BL],
                        w3_sb[c][:, t_ * 128 : (t_ + 1) * 128],
                        v_both[c][:],
                        start=True,
                        stop=True,
                    )
                prod = work.tile([128, RT * BL], BF, tag="prod", name="prod")
                nc.vector.tensor_mul(
                    prod[:], u2_all[:].rearrange("p t b -> p (t b)"), pc_all[:]
                )
                dcat = work.tile([128, RT], F32, tag=f"dcat{c}", name=f"dcat{c}")
                nc.vector.tensor_reduce(
                    dcat[:],
                    prod[:].rearrange("p (t b) -> p t b", t=RT),
                    axis=mybir.AxisListType.X,
                    op=mybir.AluOpType.add,
                )
                dtp = ps1.tile([RT, 128], F32, tag="pp", bufs=2, name="dtp")
                nc.tensor.transpose(dtp[:], dcat[:], ident_f[:])
                abar = work.tile([RT, 16], F32, tag=f"abar{c}", name=f"abar{c}")
                nc.vector.tensor_reduce(
                    abar[:],
                    dtp[:].rearrange("p (r i) -> p r i", i=8),
                    axis=mybir.AxisListType.X,
                    op=mybir.AluOpType.add,
                )
                nc.sync.dma_start(
                    a_d[it][c, :].rearrange("(t j) -> t j", t=RT), abar[:]
                )
            nc.gpsimd.collective_compute(
                "AllReduce",
                mybir.AluOpType.add,
                replica_groups=rg,
                ins=[a_d[it][:]],
                outs=[ar_a[it][:]],
            )
            ld = work.tile([NCLS, 192], F32, tag=f"arld{it}", name=f"arld{it}")
            nc.sync.dma_start(ld[:], ar_a[it][:])
            b_new = work.tile([NCLS, 192], F32, tag=f"bcur{it}", name=f"bcur{it}")
            if it == 0:
                nc.vector.tensor_scalar_mul(b_new[:], ld[:], 1.0 / B)
            else:
                scaled = work.tile([NCLS, 192], F32, tag="arsc", name="arsc")
                nc.vector.tensor_scalar_mul(scaled[:], ld[:], 1.0 / B)
                nc.vector.tensor_add(b_new[:], b_cur[:], scaled[:])
            b_cur = b_new

    # final output: y[b, (c,o)] via PE transposes of v_c
    ob = work.tile([BL, 128], F32, tag="ob", bufs=2, name="ob")
    for c in range(NCLS):
        vt = ps1.tile([BL, OC], F32, tag="pp", bufs=2, name="vt")
        nc.tensor.transpose(vt[:], v_both[c][:], ident_f[:OC, :OC])
        nc.vector.tensor_copy(ob[:, c * OC : (c + 1) * OC], vt[:])
    nc.sync.dma_start(y[:, :], ob[:])
    ps1_cm.__exit__(None, None, None)


def _build_program():
    dt1 = BF if PHASE1_DT == "bf16" else F32
    nc = bacc.Bacc(num_devices=N_CORES)

    dlog_t = nc.declare_dram_parameter("dlog_t", [KC, B], dt1, isOutput=False)
    wp = nc.declare_dram_parameter("wp", [KC, E], dt1, isOutput=False)
    beta = nc.declare_dram_parameter("beta", [E], F32, isOutput=False)
    img_t = nc.declare_dram_parameter("img_t", [E, BL], BF, isOutput=False)
    capt_t = nc.declare_dram_parameter("capt_t", [E, BL], BF, isOutput=False)
    wm2 = nc.declare_dram_parameter("wm2", [3, E, 512], BF, isOutput=False)
    bias3 = nc.declare_dram_parameter("bias3", [3, 512], BF, isOutput=False)
    w2 = nc.declare_dram_parameter("w2", [RI, 128], BF, isOutput=False)
    w3 = nc.declare_dram_parameter("w3", [128, RI], BF, isOutput=False)
    y = nc.declare_dram_parameter("y", [BL, 128], F32, isOutput=True)
    io = (dlog_t, wp, beta, img_t, capt_t, wm2, bias3, w2, w3, y)

    with tile.TileContext(nc) as tc:
        with (
            tc.tile_pool(name="const", bufs=1) as const,
            tc.tile_pool(name="loads", bufs=3) as loads,
            tc.tile_pool(name="work", bufs=2) as work,
            tc.tile_pool(name="dram", bufs=1, space="DRAM") as dram,
        ):
            _emit(nc, tc, const, loads, work, dram, io)

    nc.compile()
    return nc


def _host_prep(inputs):
    """Numpy-side sharding/layout prep. Returns per-core input maps."""
    img_emb = np.asarray(inputs["img_emb"], dtype=np.float32)
    capt_emb = np.asarray(inputs["capt_emb"], dtype=np.float32)
    dct = np.asarray(inputs["DCT_features"], dtype=np.float32).reshape(B, K)
    w_emb = np.asarray(inputs["W_emb"], dtype=np.float32)
    b_emb = np.asarray(inputs["b_emb"], dtype=np.float32)
    w_digit = np.asarray(inputs["W_digit"], dtype=np.float32)

    dlog = np.log(np.abs(dct) + 1e-12)
    mu = float(dlog.mean(dtype=np.float64))
    sigma = float(dlog.std(ddof=1, dtype=np.float64))
    s_w = w_emb.sum(axis=1, dtype=np.float64)
    beta = (b_emb - (mu / sigma) * s_w).astype(np.float32)

    np_dt1 = _BF16 if PHASE1_DT == "bf16" else np.float32
    dlog_T = np.ascontiguousarray(dlog.T).astype(np_dt1)  # [K, B]
    wp = np.ascontiguousarray(w_emb.T / sigma).astype(np_dt1)  # [K, E]

    wm2 = np.stack(
        [
            np.ascontiguousarray(
                np.asarray(inputs[f"W_{m}"], dtype=np.float32).transpose(2, 1, 0)
            ).reshape(E, 512)
            for m in ("img", "capt", "dct")
        ]
    ).astype(_BF16)  # [3, E, 512]
    bias3 = np.stack(
        [
            np.ascontiguousarray(
                np.asarray(inputs[f"b_{m}"], dtype=np.float32).T
            ).reshape(512)
            for m in ("img", "capt", "dct")
        ]
    ).astype(_BF16)  # [3, 512]
    w2 = (
        np.ascontiguousarray(w_digit.transpose(0, 3, 1, 2))
        .reshape(RI, 128)
        .astype(_BF16)
    )
    w3 = np.concatenate(
        [
            np.ascontiguousarray(w_digit[:, c].transpose(1, 0, 2)).reshape(OC, RI)
            for c in range(NCLS)
        ]
    ).astype(_BF16)  # [128, RI]
    img_T = np.ascontiguousarray(img_emb.T).astype(_BF16)  # [E, B]
    capt_T = np.ascontiguousarray(capt_emb.T).astype(_BF16)

    in_maps = []
    for c in range(N_CORES):
        in_maps.append(
            {
                "dlog_t": np.ascontiguousarray(dlog_T[c * KC : (c + 1) * KC]),
                "wp": np.ascontiguousarray(wp[c * KC : (c + 1) * KC]),
                "beta": beta,
                "img_t": np.ascontiguousarray(img_T[:, c * BL : (c + 1) * BL]),
                "capt_t": np.ascontiguousarray(capt_T[:, c * BL : (c + 1) * BL]),
                "wm2": wm2,
                "bias3": bias3,
                "w2": w2,
                "w3": w3,
            }
        )
    return in_maps


def kernel(**inputs) -> np.ndarray:
    if "nc" not in _CACHE:
        _CACHE["nc"] = _build_program()
    nc = _CACHE["nc"]
    in_maps = _host_prep(inputs)
    trace = bool(int(os.environ.get("CAPS_TRACE", "0")))
    res = run_bass_kernel_spmd(nc, in_maps, list(range(N_CORES)), trace=trace)
    _CACHE["last_result"] = res
    out = np.concatenate([r["y"] for r in res.results], axis=0).reshape(
        B, NCLS, OC
    )
    return np.ascontiguousarray(out)[:, :, :, None]


# revision 20
# speedup vs baseline: 1.1296x; 1.0006x over previous
"""CapsNet Trainium2 kernel: 8-core SPMD Bass/Tile implementation.

Strategy (v3):
  Phase 1 (contraction-parallel): the dominant op is
     dct_emb = relu(norm(log|DCT|) @ W_emb.T + b_emb),  [512,102400]x[102400,768]
  Each core owns a 12800-wide slice of the 102400 contraction dim
  (~128us of bf16 PE time — the roofline). log|x| is precomputed on
  host (needed there for the global mean/std anyway) and the affine
  normalization folds into the matmul epilogue. Phase-1 streaming loads
  own the Sync DMA queue; constant loads ride other engine queues.

  Phase 2 (batch-parallel): the per-core partial G^T[768,512] products
  are combined with a bf16 ReduceScatter over the batch dim (DRAM
  layout [8, 768, 64] so rank r receives exactly its 64 batch columns).
  Each core runs primary caps + dynamic routing for its own 64 rows;
  the two batch-mean agreement reductions are tiny [2,192] AllReduces.
  The host concatenates the per-core [64,128] outputs.

  Dynamic routing never materializes u_hat[B,192,2,64]:
     s_c   = (W2 * c)^T @ u           (contraction over (r,i)=1536)
     P_c   = W3_c^T @ v_c             (back-projected v)
     a_rc  = sum_b sum_i u * P_c      (agreement, mul+reduce)
  The elementwise digit squash uses s*|s|/(1+s^2) (the 1e-7 eps only
  matters at |s|~3e-4 where the output is ~1e-11 — far below
  tolerance), so routing needs no scalar-engine table switches.
"""

import os
import sys

import numpy as np

if "/opt/trn_rl_repo" not in sys.path:
    sys.path.insert(0, "/opt/trn_rl_repo")

import concourse.bass as bass  # noqa: E402
import concourse.mybir as mybir  # noqa: E402
import concourse.tile as tile  # noqa: E402
from concourse import bacc  # noqa: E402
from concourse.bass_utils import run_bass_kernel_spmd  # noqa: E402
from concourse.masks import make_identity  # noqa: E402

try:
    import ml_dtypes  # noqa: E402

    _BF16 = ml_dtypes.bfloat16
except Exception:  # pragma: no cover
    _BF16 = None

N_CORES = 8
B = 512  # global batch
BL = B // N_CORES  # 64 local batch rows per core
K, KC = 102400, 12800  # contraction dim, per-core slice
E = 768  # embedding
ET = E // 128  # e chunks
KT = KC // 128  # k tiles per core (100)
GROUP = 10  # k tiles per load DMA
RI = 1536  # (route, in_cap) flat = 192*8
RT = RI // 128  # 12 tiles
NCLS = 2
OC = 64  # out caps channels
F32 = mybir.dt.float32
BF = mybir.dt.bfloat16

PHASE1_DT = os.environ.get("CAPS_P1_DT", "bf16")
RS_DT = os.environ.get("CAPS_RS_DT", "bf16")
# bisection: 1=phase1 only, 2=+RS, 3=+prim/squash/u2, 5=full
STOP = int(os.environ.get("CAPS_STOP", "5"))

_CACHE = {}


def _emit(nc, tc, const, loads, work, dram, io):
    dt1 = BF if PHASE1_DT == "bf16" else F32
    dt_rs = BF if RS_DT == "bf16" else F32
    rg = [list(range(N_CORES))]
    dlog_t, wp, beta, img_t, capt_t, wm2, bias3, w2, w3, y = io

    def debug_out(fill=None):
        out_sb = work.tile([BL, 128], F32, tag="outsb", name="outsb")
        nc.vector.memset(out_sb[:], 0.0)
        if fill is not None:
            fill(out_sb)
        nc.sync.dma_start(y[:, :], out_sb[:])

    # ---------------- phase 1: big matmul (loads on Sync queue) --------
    psA_cm = tc.tile_pool(name="psA", bufs=1, space="PSUM")
    psA = psA_cm.__enter__()
    g_ps = [psA.tile([128, B], F32, tag=f"g{ec}", name=f"g{ec}") for ec in range(ET)]
    n_loads = KT // GROUP

    def _emit_phase1():
        groups = [2, 3, 5] + [GROUP] * ((KT - 10) // GROUP)
        assert sum(groups) == KT
        kt = 0
        for g in groups:
            k0 = kt * 128
            dlog = loads.tile([128, g, B], dt1, tag="dlog")
            nc.sync.dma_start(
                dlog[:],
                dlog_t[k0 : k0 + g * 128, :].rearrange("(s p) b -> p s b", p=128),
            )
            w_tile = loads.tile([128, g, E], dt1, tag="w")
            nc.sync.dma_start(
                w_tile[:],
                wp[k0 : k0 + g * 128, :].rearrange("(s p) e -> p s e", p=128),
            )
            for s in range(g):
                for ec in range(ET):
                    nc.tensor.matmul(
                        g_ps[ec][:],
                        w_tile[:, s, ec * 128 : (ec + 1) * 128],
                        dlog[:, s, :],
                        start=(kt == 0),
                        stop=(kt == KT - 1),
                    )
                kt += 1

    # ---------------- constants (DMA on vector/scalar/tensor queues) ----
    eps_sq = const.tile([128, 1], F32)
    nc.vector.memset(eps_sq[:], 1e-7)
    ident_bf = const.tile([128, 128], BF)
    make_identity(nc, ident_bf[:])
    ident_f = const.tile([128, 128], F32)
    make_identity(nc, ident_f[:])
    beta_sb = const.tile([128, ET], F32)
    emb_sb = {}  # (m, et) -> [128, BL] bf16 tile (lhsT source for prim)
    wm2_sb = {}
    bias_sb = []
    w2_sb = []
    w3_sb = []
    with tc.tile_wait_until(0.115):
        nc.gpsimd.dma_start(beta_sb[:], beta[:].rearrange("(t p) -> p t", p=128))
        for m, src in ((0, img_t), (1, capt_t)):
            for et in range(ET):
                t = const.tile(
                    [128, BL], BF, tag=f"emb{m}_{et}", name=f"emb{m}_{et}"
                )
                nc.gpsimd.dma_start(t[:], src[et * 128 : (et + 1) * 128, :])
                emb_sb[(m, et)] = t
        for m in range(3):
            for et in range(ET):
                t = const.tile(
                    [128, 512], BF, tag=f"wm2_{m}_{et}", name=f"wm2_{m}_{et}"
                )
                nc.scalar.dma_start(t[:], wm2[m, et * 128 : (et + 1) * 128, :])
                wm2_sb[(m, et)] = t
        for m in range(3):
            t = const.tile([1, 512], BF, tag=f"bias{m}", name=f"bias{m}")
            nc.scalar.dma_start(t[:], bias3[m : m + 1, :])
            bias_sb.append(t)
    ones1 = const.tile([1, BL], BF)
    nc.vector.memset(ones1[:], 1.0)
    with tc.tile_wait_until(0.15):
        for t_ in range(RT):
            t = const.tile([128, 128], BF, tag=f"w2_{t_}", name=f"w2_{t_}")
            nc.gpsimd.dma_start(t[:], w2[t_ * 128 : (t_ + 1) * 128, :])
            w2_sb.append(t)
        for c in range(NCLS):
            t = const.tile([OC, RI], BF, tag=f"w3_{c}", name=f"w3_{c}")
            nc.gpsimd.dma_start(t[:], w3[c * OC : (c + 1) * OC, :])
            w3_sb.append(t)

    _emit_phase1()

    # evacuate PSUM -> SBUF -> cc_in[8, 768, 64]; ReduceScatter over batch
    cc_in = dram.tile([N_CORES, E, BL], dt_rs)
    for ec in range(ET):
        g_sb = work.tile([128, B], dt_rs, tag="gsb", bufs=2)
        if ec % 2 == 0:
            nc.vector.tensor_copy(g_sb[:], g_ps[ec][:])
        else:
            nc.scalar.copy(g_sb[:], g_ps[ec][:])
        nc.sync.dma_start(
            cc_in[:, ec * 128 : (ec + 1) * 128, :].rearrange("r p b -> p r b"),
            g_sb[:].rearrange("p (r b) -> p r b", r=N_CORES),
        )
    psA_cm.__exit__(None, None, None)
    ps1_cm = tc.tile_pool(name="psB", bufs=1, space="PSUM")
    ps1 = ps1_cm.__enter__()
    if STOP == 1:
        debug_out()
        return

    rs_out = dram.tile([E, BL], dt_rs)
    nc.gpsimd.collective_compute(
        "ReduceScatter",
        mybir.AluOpType.add,
        replica_groups=rg,
        ins=[cc_in[:]],
        outs=[rs_out[:]],
    )
    if STOP == 2:
        debug_out(lambda o: nc.sync.dma_start(o[:, :64], rs_out[:BL, :]))
        return

    # ---------------- primary caps + squash + transpose to u2 ----------
    u2_all = const.tile([128, RT, BL], BF)  # [(r,i)-tile, t, b]

    def prim_chain(m):
        pm = ps1.tile([BL, 512], F32, tag="pp", bufs=2, name="pm")
        for et in range(ET):
            nc.tensor.matmul(
                pm[:],
                emb_sb[(m, et)][:],
                wm2_sb[(m, et)][:],
                start=(et == 0),
                stop=False,
            )
        nc.tensor.matmul(pm[:], ones1[:], bias_sb[m][:], start=False, stop=True)
        upre_c = work.tile([BL, 512], F32, tag="upre", bufs=3, name="upre")
        nc.vector.tensor_copy(upre_c[:], pm[:])
        # squash over i (groups of 8 in the free dim), 64 routes here
        sq8 = work.tile([BL, 512], F32, tag="sq8", bufs=2)
        nc.vector.tensor_mul(sq8[:], upre_c[:], upre_c[:])
        usq = work.tile([BL, 64], F32, tag="usq")
        nc.vector.tensor_reduce(
            usq[:],
            sq8[:].rearrange("p (r i) -> p r i", i=8),
            axis=mybir.AxisListType.X,
            op=mybir.AluOpType.add,
        )
        t1 = work.tile([BL, 64], F32, tag="fa")
        nc.scalar.activation(
            t1[:], usq[:], mybir.ActivationFunctionType.Sqrt, bias=eps_sq[:BL, :]
        )
        t2 = work.tile([BL, 64], F32, tag="fb")
        nc.vector.tensor_scalar_add(t2[:], usq[:], 1.0)
        t3 = work.tile([BL, 64], F32, tag="fc")
        nc.vector.tensor_mul(t3[:], t1[:], t2[:])
        t4 = work.tile([BL, 64], F32, tag="fd")
        nc.vector.reciprocal_approx_fast(t4[:], t3[:])
        t5 = work.tile([BL, 64], F32, tag="fe")
        nc.vector.tensor_mul(t5[:], t4[:], usq[:])
        usquash = work.tile([BL, 512], BF, tag="usquash", bufs=2)
        nc.vector.tensor_tensor(
            usquash[:].rearrange("p (r i) -> p r i", i=8),
            upre_c[:].rearrange("p (r i) -> p r i", i=8),
            t5[:].broadcast_to([BL, 64, 8]),
            op=mybir.AluOpType.mult,
        )
        for j in range(4):
            tp = ps1.tile([128, BL], BF, tag="pp", bufs=2, name="tp")
            nc.tensor.transpose(
                tp[:], usquash[:, j * 128 : (j + 1) * 128], ident_bf[:BL, :BL]
            )
            nc.vector.tensor_copy(u2_all[:, 4 * m + j, :], tp[:])

    # img/capt chains execute on the PE during the ReduceScatter
    prim_chain(0)
    prim_chain(1)

    # iter-0 s-matmul partial accumulation over img/capt tiles (RS shadow);
    # both classes packed on partitions: s_ps[(c,o), b]
    s_ps = ps1.tile([128, BL], F32, tag="sps", bufs=2, name="sps")
    for t_ in range(8):
        nc.tensor.matmul(
            s_ps[:],
            w2_sb[t_][:],
            u2_all[:, t_, :],
            start=(t_ == 0),
            stop=False,
        )

    # dct embedding tiles: relu(rs_out + beta) -> bf16 (vector, no tables)
    gp = work.tile([128, ET, BL], dt_rs, tag="gp")
    nc.sync.dma_start(gp[:], rs_out[:].rearrange("(t p) b -> p t b", p=128))
    for et in range(ET):
        t = const.tile([128, BL], BF, tag=f"emb2_{et}", name=f"emb2_{et}")
        nc.vector.tensor_scalar(
            t[:],
            gp[:, et, :],
            beta_sb[:, et : et + 1],
            0.0,
            op0=mybir.AluOpType.add,
            op1=mybir.AluOpType.max,
        )
        emb_sb[(2, et)] = t
    prim_chain(2)
    if STOP == 3:
        debug_out(
            lambda o: nc.vector.tensor_copy(
                o[:, :64], u2_all[:BL, 0, :]
            )
        )
        return

    # ---------------- dynamic routing (batch-sharded) -------------------
    a_d = [dram.tile([NCLS, 192], F32, name=f"a_d{i}") for i in range(2)]
    ar_a = [
        dram.tile([NCLS, 192], F32, addr_space="Shared", name=f"ar_a{i}")
        for i in range(2)
    ]
    c_dram = [dram.tile([192, NCLS], F32, name=f"c_dram{i}") for i in range(2)]
    b_cur = None  # [2,192] logits tile

    v_both = None
    for it in range(3):
        rnd = it - 1
        if it == 0:
            mset = w2_sb  # uniform c folded into evac scale 1/192
        else:
            # softmax(b_cur) over routes -> c_sm [2,192]
            mx = work.tile([NCLS, 1], F32, tag="smx", name="smx")
            nc.vector.tensor_reduce(
                mx[:], b_cur[:], axis=mybir.AxisListType.X, op=mybir.AluOpType.max
            )
            mxn = work.tile([NCLS, 1], F32, tag="smxn", name="smxn")
            nc.vector.tensor_scalar_mul(mxn[:], mx[:], -1.0)
            ex = work.tile([NCLS, 192], F32, tag="sex", name="sex")
            nc.scalar.activation(
                ex[:], b_cur[:], mybir.ActivationFunctionType.Exp, bias=mxn[:]
            )
            sm = work.tile([NCLS, 1], F32, tag="ssm", name="ssm")
            nc.vector.tensor_reduce(
                sm[:], ex[:], axis=mybir.AxisListType.X, op=mybir.AluOpType.add
            )
            rcp = work.tile([NCLS, 1], F32, tag="srcp", name="srcp")
            nc.vector.reciprocal(rcp[:], sm[:])
            c_sm = work.tile([NCLS, 192], F32, tag="scs", name="scs")
            nc.vector.tensor_scalar(
                c_sm[:], ex[:], rcp[:], None, op0=mybir.AluOpType.mult
            )
            # c [2,192] -> DRAM [192,2] -> broadcast-read c_exp [128, RT, 2]
            nc.sync.dma_start(c_dram[rnd][:].rearrange("r c -> c r"), c_sm[:])
            c_exp = work.tile([128, RT, NCLS], F32, tag="cexp", name="cexp")
            for t_ in range(RT):
                eng = (nc.sync, nc.scalar)[t_ % 2]
                eng.dma_start(
                    c_exp[:, t_, :],
                    c_dram[rnd][16 * t_ : 16 * (t_ + 1), :]
                    .broadcast_to([16, NCLS, 8])
                    .rearrange("j c r -> j r c"),
                )
            # mset[t] = w2[t] * c  (both classes in one op; vector/gpsimd)
            mset = []
            for t_ in range(RT):
                msc = work.tile([128, 128], BF, tag="msc", bufs=12, name="msc")
                nc.vector.tensor_tensor(
                    msc[:].rearrange("p (c o) -> p c o", c=NCLS),
                    w2_sb[t_][:].rearrange("p (c o) -> p c o", c=NCLS),
                    c_exp[:, t_, :].broadcast_to([128, NCLS, OC]),
                    op=mybir.AluOpType.mult,
                )
                mset.append(msc)

        # s for both classes packed on partitions: one matmul chain
        if it > 0:
            s_ps = ps1.tile([128, BL], F32, tag="sps", bufs=2, name="sps")
        t_start = 8 if it == 0 else 0
        for t_ in range(t_start, RT):
            nc.tensor.matmul(
                s_ps[:],
                mset[t_][:],
                u2_all[:, t_, :],
                start=(it > 0 and t_ == 0),
                stop=(t_ == RT - 1),
            )
        s_sb = work.tile([128, BL], F32, tag="ssb", name="ssb")
        nc.vector.tensor_scalar_mul(
            s_sb[:], s_ps[:], (1.0 / 192.0) if it == 0 else 1.0
        )
        # elementwise digit squash v = s*|s|/(1+s^2) on [(c,o), BL]
        sq = work.tile([128, BL], F32, tag="dsq", name="dsq")
        nc.vector.tensor_mul(sq[:], s_sb[:], s_sb[:])
        den = work.tile([128, BL], F32, tag="dd2", name="dd2")
        nc.vector.tensor_scalar_add(den[:], sq[:], 1.0)
        rec = work.tile([128, BL], F32, tag="dd4", name="dd4")
        nc.vector.reciprocal_approx_fast(rec[:], den[:])
        ns = work.tile([128, BL], F32, tag="dn", name="dn")
        nc.vector.tensor_scalar_mul(ns[:], s_sb[:], -1.0)
        sab = work.tile([128, BL], F32, tag="dd1", name="dd1")
        nc.vector.tensor_tensor(sab[:], s_sb[:], ns[:], op=mybir.AluOpType.max)
        num = work.tile([128, BL], F32, tag="dd3", name="dd3")
        nc.vector.tensor_mul(num[:], sab[:], s_sb[:])
        vv = work.tile(
            [128, BL],
            BF if it < 2 else F32,
            tag=f"vb{'f' if it == 2 else ''}",
            name="vv",
            bufs=2,
        )
        nc.vector.tensor_mul(vv[:], num[:], rec[:])
        v_both = vv

        if it < 2:
            # agreement: abar[(c,r)] = sum_b sum_i u2 * (W3_c^T @ v_c)
            v_hi = work.tile([OC, BL], BF, tag="vhi", name="v_hi")
            nc.vector.tensor_copy(v_hi[:], v_both[OC:, :])
            pc_all = ps1.tile(
                [128, NCLS * RT * BL], F32, tag="pca", bufs=1, name="pca"
            )
            for c in range(NCLS):
                rhs = v_both[:OC, :] if c == 0 else v_hi[:]
                for t_ in range(RT):
                    nc.tensor.matmul(
                        pc_all[:, (c * RT + t_) * BL : (c * RT + t_ + 1) * BL],
                        w3_sb[c][:, t_ * 128 : (t_ + 1) * 128],
                        rhs,
                        start=True,
                        stop=True,
                    )
            prod = work.tile([128, NCLS * RT * BL], BF, tag="prod", name="prod")
            for c in range(NCLS):
                nc.vector.tensor_mul(
                    prod[:, c * RT * BL : (c + 1) * RT * BL],
                    u2_all[:].rearrange("p t b -> p (t b)"),
                    pc_all[:, c * RT * BL : (c + 1) * RT * BL],
                )
            dcat = work.tile([128, NCLS * RT], F32, tag="dcat", name="dcat")
            nc.vector.tensor_reduce(
                dcat[:],
                prod[:].rearrange("p (ct b) -> p ct b", b=BL),
                axis=mybir.AxisListType.X,
                op=mybir.AluOpType.add,
            )
            dtp = ps1.tile([NCLS * RT, 128], F32, tag="pp", bufs=2, name="dtp")
            nc.tensor.transpose(dtp[:], dcat[:], ident_f[:])
            abar = work.tile([NCLS * RT, 16], F32, tag="abar", name="abar")
            nc.vector.tensor_reduce(
                abar[:],
                dtp[:].rearrange("p (r i) -> p r i", i=8),
                axis=mybir.AxisListType.X,
                op=mybir.AluOpType.add,
            )
            for c in range(NCLS):
                nc.sync.dma_start(
                    a_d[it][c, :].rearrange("(t j) -> t j", t=RT),
                    abar[c * RT : (c + 1) * RT, :],
                )
            nc.gpsimd.collective_compute(
                "AllReduce",
                mybir.AluOpType.add,
                replica_groups=rg,
                ins=[a_d[it][:]],
                outs=[ar_a[it][:]],
            )
            ld = work.tile([NCLS, 192], F32, tag=f"arld{it}", name=f"arld{it}")
            nc.sync.dma_start(ld[:], ar_a[it][:])
            b_new = work.tile([NCLS, 192], F32, tag=f"bcur{it}", name=f"bcur{it}")
            if it == 0:
                nc.vector.tensor_scalar_mul(b_new[:], ld[:], 1.0 / B)
            else:
                scaled = work.tile([NCLS, 192], F32, tag="arsc", name="arsc")
                nc.vector.tensor_scalar_mul(scaled[:], ld[:], 1.0 / B)
                nc.vector.tensor_add(b_new[:], b_cur[:], scaled[:])
            b_cur = b_new

    # final output: y[b, (c,o)] via one PE transpose of packed v
    vt = ps1.tile([BL, 128], F32, tag="pp", bufs=2, name="vt")
    nc.tensor.transpose(vt[:], v_both[:], ident_f[:])
    ob = work.tile([BL, 128], F32, tag="ob", bufs=2, name="ob")
    nc.vector.tensor_copy(ob[:], vt[:])
    nc.sync.dma_start(y[:, :], ob[:])
    ps1_cm.__exit__(None, None, None)


def _build_program():
    dt1 = BF if PHASE1_DT == "bf16" else F32
    nc = bacc.Bacc(num_devices=N_CORES)

    dlog_t = nc.declare_dram_parameter("dlog_t", [KC, B], dt1, isOutput=False)
    wp = nc.declare_dram_parameter("wp", [KC, E], dt1, isOutput=False)
    beta = nc.declare_dram_parameter("beta", [E], F32, isOutput=False)
    img_t = nc.declare_dram_parameter("img_t", [E, BL], BF, isOutput=False)
    capt_t = nc.declare_dram_parameter("capt_t", [E, BL], BF, isOutput=False)
    wm2 = nc.declare_dram_parameter("wm2", [3, E, 512], BF, isOutput=False)
    bias3 = nc.declare_dram_parameter("bias3", [3, 512], BF, isOutput=False)
    w2 = nc.declare_dram_parameter("w2", [RI, 128], BF, isOutput=False)
    w3 = nc.declare_dram_parameter("w3", [128, RI], BF, isOutput=False)
    y = nc.declare_dram_parameter("y", [BL, 128], F32, isOutput=True)
    io = (dlog_t, wp, beta, img_t, capt_t, wm2, bias3, w2, w3, y)

    with tile.TileContext(nc) as tc:
        with (
            tc.tile_pool(name="const", bufs=1) as const,
            tc.tile_pool(name="loads", bufs=3) as loads,
            tc.tile_pool(name="work", bufs=2) as work,
            tc.tile_pool(name="dram", bufs=1, space="DRAM") as dram,
        ):
            _emit(nc, tc, const, loads, work, dram, io)

    nc.compile()
    return nc


def _host_prep(inputs):
    """Numpy-side sharding/layout prep. Returns per-core input maps."""
    img_emb = np.asarray(inputs["img_emb"], dtype=np.float32)
    capt_emb = np.asarray(inputs["capt_emb"], dtype=np.float32)
    dct = np.asarray(inputs["DCT_features"], dtype=np.float32).reshape(B, K)
    w_emb = np.asarray(inputs["W_emb"], dtype=np.float32)
    b_emb = np.asarray(inputs["b_emb"], dtype=np.float32)
    w_digit = np.asarray(inputs["W_digit"], dtype=np.float32)

    dlog = np.log(np.abs(dct) + 1e-12)
    mu = float(dlog.mean(dtype=np.float64))
    sigma = float(dlog.std(ddof=1, dtype=np.float64))
    s_w = w_emb.sum(axis=1, dtype=np.float64)
    beta = (b_emb - (mu / sigma) * s_w).astype(np.float32)

    np_dt1 = _BF16 if PHASE1_DT == "bf16" else np.float32
    dlog_T = np.ascontiguousarray(dlog.T).astype(np_dt1)  # [K, B]
    wp = np.ascontiguousarray(w_emb.T / sigma).astype(np_dt1)  # [K, E]

    wm2 = np.stack(
        [
            np.ascontiguousarray(
                np.asarray(inputs[f"W_{m}"], dtype=np.float32).transpose(2, 1, 0)
            ).reshape(E, 512)
            for m in ("img", "capt", "dct")
        ]
    ).astype(_BF16)  # [3, E, 512]
    bias3 = np.stack(
        [
            np.ascontiguousarray(
                np.asarray(inputs[f"b_{m}"], dtype=np.float32).T
            ).reshape(512)
            for m in ("img", "capt", "dct")
        ]
    ).astype(_BF16)  # [3, 512]
    w2 = (
        np.ascontiguousarray(w_digit.transpose(0, 3, 1, 2))
        .reshape(RI, 128)
        .astype(_BF16)
    )
    w3 = np.concatenate(
        [
            np.ascontiguousarray(w_digit[:, c].transpose(1, 0, 2)).reshape(OC, RI)
            for c in range(NCLS)
        ]
    ).astype(_BF16)  # [128, RI]
    img_T = np.ascontiguousarray(img_emb.T).astype(_BF16)  # [E, B]
    capt_T = np.ascontiguousarray(capt_emb.T).astype(_BF16)

    in_maps = []
    for c in range(N_CORES):
        in_maps.append(
            {
                "dlog_t": np.ascontiguousarray(dlog_T[c * KC : (c + 1) * KC]),
                "wp": np.ascontiguousarray(wp[c * KC : (c + 1) * KC]),
                "beta": beta,
                "img_t": np.ascontiguousarray(img_T[:, c * BL : (c + 1) * BL]),
                "capt_t": np.ascontiguousarray(capt_T[:, c * BL : (c + 1) * BL]),
                "wm2": wm2,
                "bias3": bias3,
                "w2": w2,
                "w3": w3,
            }
        )
    return in_maps


def kernel(**inputs) -> np.ndarray:
    if "nc" not in _CACHE:
        _CACHE["nc"] = _build_program()
    nc = _CACHE["nc"]
    in_maps = _host_prep(inputs)
    trace = bool(int(os.environ.get("CAPS_TRACE", "0")))
    res = run_bass_kernel_spmd(nc, in_maps, list(range(N_CORES)), trace=trace)
    _CACHE["last_result"] = res
    out = np.concatenate([r["y"] for r in res.results], axis=0).reshape(
        B, NCLS, OC
    )
    return np.ascontiguousarray(out)[:, :, :, None]
